# revision 4
# baseline (speedup 1.0000x reference)
"""Trainium2 Bass kernel for nn_Att_mlp_softmax (GNN message passing).

Reference computation:
    e = relu(h @ W1 + b1) @ W2 + b2                       # [N, 1] per-node score
    att = softmax(where(G > 0, e.T broadcast, -9e15))     # row-wise over neighbors
    out = (G.sum(-1))[:, None] * (att @ h)                # degree-rescaled aggregation

Because the pre-softmax score of entry (i, j) depends only on column j, the
masked softmax collapses algebraically:
    att[i, j] = G[i, j] * w[j] / sum_j G[i, j] * w[j],  w = exp(e - c)
so with H' = [w * h | w | 1] (N x 130):
    Y = G @ H'
    out = Y[:, 129] * Y[:, :128] / Y[:, 128]
One big [N, N] x [N, 130] matmul replaces the N^2 softmax entirely.

Distribution: G is row-sharded across 8 NeuronCores (1024 rows each); h and the
MLP weights are replicated. Each core's G shard is passed pre-transposed
([N, 1024], contraction dim major) so SBUF tiles have the contraction dim on
partitions with fully contiguous 4 KB DMA lines. No collectives needed.
"""

import numpy as np

N = 8192
D = 128
HID = 64
N_CORES = 8
ROWS = N // N_CORES          # 1024 output rows per core
JC = N // 128                # 64 contraction chunks of 128
ESHIFT = -4.0                # exp(e - 4): cancels exactly in the ratio, keeps
                             # w in a comfortable fp32 range

_cache = {}


def _install_axon_hooks_shim():
    """Provide antenv.axon_hooks if the image lacks it (trn_boot step 6).

    concourse.bass_utils imports it unconditionally when BASS_TRACE is set;
    without the shim that import crashes instead of degrading.
    """
    import contextlib
    import ctypes
    import sys
    import types

    try:
        import antenv.axon_hooks  # noqa: F401
        return
    except ImportError:
        pass

    so_path = "/opt/axon/libaxon_pjrt.so"

    def _make_hook():
        try:
            lib = ctypes.CDLL(so_path)
        except OSError:
            return None
        if not hasattr(lib, "axon_start_nrt_profile"):
            return None
        lib.axon_start_nrt_profile.argtypes = [
            ctypes.POINTER(ctypes.c_int64),
            ctypes.c_size_t,
        ]
        lib.axon_start_nrt_profile.restype = ctypes.c_int64
        lib.axon_stop_nrt_profile.argtypes = [ctypes.c_char_p]
        lib.axon_stop_nrt_profile.restype = ctypes.c_int64

        @contextlib.contextmanager
        def _hook(output_dir, device_ids):
            import jax

            jax.devices()
            if device_ids:
                ids = (ctypes.c_int64 * len(device_ids))(*device_ids)
                rc = lib.axon_start_nrt_profile(ids, len(device_ids))
            else:
                rc = lib.axon_start_nrt_profile(None, 0)
            if rc != 0:
                raise RuntimeError(f"axon_start_nrt_profile rc={rc}")
            try:
                yield
            finally:
                lib.axon_stop_nrt_profile(str(output_dir).encode())

        return _hook

    mod = types.ModuleType("antenv.axon_hooks")
    _holder = {"hook": _make_hook()}
    mod.set_axon_ntff_profile_hook = lambda h: _holder.__setitem__("hook", h)
    mod.get_axon_ntff_profile_hook = lambda: _holder["hook"]
    sys.modules["antenv.axon_hooks"] = mod
    try:
        import antenv

        antenv.axon_hooks = mod
    except ImportError:
        pass


def build_nc(enable_asserts=False):
    """Build + compile the per-core Bass program (identical on all 8 cores)."""
    from concourse import bacc, mybir, tile
    from concourse.masks import make_identity

    f32 = mybir.dt.float32
    AF = mybir.ActivationFunctionType

    nc = bacc.Bacc(
        "TRN2",
        target_bir_lowering=False,
        debug=False,
        enable_asserts=enable_asserts,
        num_devices=N_CORES,
    )
    gT = nc.dram_tensor("gT", [N, ROWS], f32, kind="ExternalInput").ap()
    hT = nc.dram_tensor("hT", [D, N], f32, kind="ExternalInput").ap()
    W1 = nc.dram_tensor("W1", [D, HID], f32, kind="ExternalInput").ap()
    b1 = nc.dram_tensor("b1", [HID, 1], f32, kind="ExternalInput").ap()
    W2b = nc.dram_tensor("W2b", [HID + 1, 1], f32, kind="ExternalInput").ap()
    out = nc.dram_tensor("out", [ROWS, D], f32, kind="ExternalOutput").ap()

    with tile.TileContext(nc) as tc:
        with (
            tc.tile_pool(name="const", bufs=1) as cpool,
            tc.tile_pool(name="big", bufs=1) as bigpool,
            tc.tile_pool(name="gbuf", bufs=3) as gpool,
            tc.tile_pool(name="outbuf", bufs=3) as opool,
            tc.tile_pool(name="small", bufs=2) as spool,
        ):
            W1_sb = cpool.tile([D, HID], f32)
            nc.sync.dma_start(W1_sb[:], W1[:])
            b1_sb = cpool.tile([HID, 1], f32)
            nc.sync.dma_start(b1_sb[:], b1[:])
            W2b_sb = cpool.tile([HID + 1, 1], f32)
            nc.sync.dma_start(W2b_sb[:], W2b[:])
            ident = cpool.tile([128, 128], f32)
            make_identity(nc, ident[:])
            eshift_sb = cpool.tile([128, 1], f32)
            nc.vector.memset(eshift_sb[:], ESHIFT)

            hT_sb = bigpool.tile([D, N], f32)
            nc.sync.dma_start(hT_sb[:], hT[:])

            # aT rows 0..63 hold relu(W1.T @ hT + b1); row 64 is ones so that
            # [aT; 1].T @ [W2; b2] fuses the b2 add into the score matmul.
            aT = bigpool.tile([HID + 1, N], f32)
            nc.vector.memset(aT[HID : HID + 1, :], 1.0)

            w_sb = cpool.tile([128, JC], f32)
            # H' chunks: [:, jc, 0:128] = w * h chunk, 128 = w, 129 = ones.
            Hp = bigpool.tile([128, JC, 130], f32)
            nc.vector.memset(Hp[:, :, 129:130], 1.0)

            with tc.tile_pool(name="ps_pre", bufs=2, space="PSUM") as ps_pre:
                for nb in range(N // 512):
                    pa = ps_pre.tile([HID, 512], f32, tag="pa")
                    nc.tensor.matmul(
                        pa[:],
                        W1_sb[:],
                        hT_sb[:, nb * 512 : (nb + 1) * 512],
                        start=True,
                        stop=True,
                    )
                    nc.scalar.activation(
                        aT[0:HID, nb * 512 : (nb + 1) * 512],
                        pa[:],
                        AF.Relu,
                        bias=b1_sb[:],
                    )

                # e laid out [128, 64]: partition = j within chunk, column = chunk.
                pe = ps_pre.tile([128, JC], f32, tag="pe")
                for c in range(JC):
                    nc.tensor.matmul(
                        pe[:, c : c + 1],
                        aT[:, c * 128 : (c + 1) * 128],
                        W2b_sb[:],
                        start=True,
                        stop=True,
                    )
                nc.scalar.activation(w_sb[:], pe[:], AF.Exp, bias=eshift_sb[:])

                for jc in range(JC):
                    pt = ps_pre.tile([128, 128], f32, tag="pt")
                    nc.tensor.transpose(
                        pt[:], hT_sb[:, jc * 128 : (jc + 1) * 128], ident[:]
                    )
                    nc.vector.tensor_scalar_mul(
                        Hp[:, jc, 0:128], pt[:], w_sb[:, jc : jc + 1]
                    )
                    nc.vector.tensor_copy(
                        Hp[:, jc, 128:129], w_sb[:, jc : jc + 1]
                    )

            # Main accumulation: acc[it] [128, 130] = G_rows @ H' for i-tile it.
            gTr = gT.rearrange("(a p) i -> p a i", p=128)  # [128, JC, ROWS]
            with tc.tile_pool(name="ps_acc", bufs=8, space="PSUM") as ps_acc:
                accs = [
                    ps_acc.tile([128, 130], f32, tag="acc", name=f"acc{i}")
                    for i in range(8)
                ]
                GRP = 4  # contraction chunks per DMA (2 MB transfers)
                for jg in range(JC // GRP):
                    gt = gpool.tile([128, GRP, ROWS], f32, tag="gt")
                    nc.sync.dma_start(
                        gt[:], gTr[:, jg * GRP : (jg + 1) * GRP, :]
                    )
                    for jci in range(GRP):
                        jc = jg * GRP + jci
                        for it in range(8):
                            nc.tensor.matmul(
                                accs[it][:],
                                gt[:, jci, it * 128 : (it + 1) * 128],
                                Hp[:, jc, :],
                                start=(jc == 0),
                                stop=(jc == JC - 1),
                            )

                for it in range(8):
                    sw = spool.tile([128, 1], f32, tag="sw")
                    nc.vector.tensor_scalar_add(sw[:], accs[it][:, 128:129], 1e-30)
                    rc = spool.tile([128, 1], f32, tag="rc")
                    nc.vector.reciprocal(rc[:], sw[:])
                    r = spool.tile([128, 1], f32, tag="r")
                    nc.vector.tensor_mul(r[:], rc[:], accs[it][:, 129:130])
                    ot = opool.tile([128, D], f32, tag="ot")
                    nc.vector.tensor_scalar_mul(ot[:], accs[it][:, 0:128], r[:])
                    nc.sync.dma_start(out[it * 128 : (it + 1) * 128, :], ot[:])

    nc.compile()
    return nc


def make_in_maps(graph_info, h, W1, b1, W2, b2):
    """Shard + lay out the full inputs for the 8 cores."""
    g = np.ascontiguousarray(graph_info, dtype=np.float32)
    GT = np.ascontiguousarray(g.T)                       # [N, N], col i-major
    hT = np.ascontiguousarray(np.asarray(h, np.float32).T)  # [D, N]
    W1 = np.ascontiguousarray(np.asarray(W1, np.float32))
    b1r = np.asarray(b1, np.float32).reshape(HID, 1)
    W2b = np.concatenate(
        [np.asarray(W2, np.float32).reshape(HID, 1),
         np.asarray(b2, np.float32).reshape(1, 1)], axis=0
    )
    in_maps = []
    for c in range(N_CORES):
        in_maps.append(
            {
                "gT": GT[:, c * ROWS : (c + 1) * ROWS],
                "hT": hT,
                "W1": W1,
                "b1": b1r,
                "W2b": W2b,
            }
        )
    return in_maps


def kernel(graph_info, h, W1, b1, W2, b2):
    _install_axon_hooks_shim()
    from concourse.bass_utils import run_bass_kernel_spmd

    if "nc" not in _cache:
        _cache["nc"] = build_nc()
    nc = _cache["nc"]

    in_maps = make_in_maps(graph_info, h, W1, b1, W2, b2)
    res = run_bass_kernel_spmd(nc, in_maps, list(range(N_CORES)))
    return np.concatenate([res.results[c]["out"] for c in range(N_CORES)], axis=0)


# revision 6
# speedup vs baseline: 1.1598x; 1.1598x over previous
"""Trainium2 Bass kernel for nn_Att_mlp_softmax (GNN message passing).

Reference computation:
    e = relu(h @ W1 + b1) @ W2 + b2                       # [N, 1] per-node score
    att = softmax(where(G > 0, e.T broadcast, -9e15))     # row-wise over neighbors
    out = (G.sum(-1))[:, None] * (att @ h)                # degree-rescaled aggregation

Because the pre-softmax score of entry (i, j) depends only on column j, the
masked softmax collapses algebraically:
    att[i, j] = G[i, j] * w[j] / sum_j G[i, j] * w[j],  w = exp(e - c)
so with H' = [w * h | w | 1] (N x 130):
    Y = G @ H'
    out = Y[:, 129] * Y[:, :128] / Y[:, 128]
One big [N, N] x [N, 130] matmul replaces the N^2 softmax entirely.

Precision/perf: G is an exact 0/1 mask, so it is streamed in bf16 losslessly
(half the HBM traffic, 1 cycle/row matmul, FWL weight loads). H' is split
hi/lo into two bf16 columns per logical column (err ~2^-16), accumulated in
fp32 PSUM — near-fp32 accuracy at bf16 speed.

Distribution: G is row-sharded across 8 NeuronCores (1024 rows each); h and the
MLP weights are replicated. Each core's G shard is passed pre-transposed
([N, 1024], contraction dim major) so SBUF tiles have the contraction dim on
partitions with fully contiguous DMA lines. No collectives needed.

Moving-operand layout per contraction chunk jc (259 bf16 columns):
    [0:128]  hi(w * h)    [128:256] lo(w * h)
    [256]    hi(w)        [257]     lo(w)        [258] ones
"""

import numpy as np

N = 8192
D = 128
HID = 64
N_CORES = 8
ROWS = N // N_CORES          # 1024 output rows per core
JC = N // 128                # 64 contraction chunks of 128
NCOL = 259                   # moving columns per chunk (hi|lo|w_hi|w_lo|1)
ESHIFT = -4.0                # exp(e - 4): cancels exactly in the ratio, keeps
                             # w in a comfortable fp32/bf16 range

_cache = {}


def _install_axon_hooks_shim():
    """Provide antenv.axon_hooks if the image lacks it (trn_boot step 6).

    concourse.bass_utils imports it unconditionally when BASS_TRACE is set;
    without the shim that import crashes instead of degrading.
    """
    import contextlib
    import ctypes
    import sys
    import types

    try:
        import antenv.axon_hooks  # noqa: F401
        return
    except ImportError:
        pass

    so_path = "/opt/axon/libaxon_pjrt.so"

    def _make_hook():
        try:
            lib = ctypes.CDLL(so_path)
        except OSError:
            return None
        if not hasattr(lib, "axon_start_nrt_profile"):
            return None
        lib.axon_start_nrt_profile.argtypes = [
            ctypes.POINTER(ctypes.c_int64),
            ctypes.c_size_t,
        ]
        lib.axon_start_nrt_profile.restype = ctypes.c_int64
        lib.axon_stop_nrt_profile.argtypes = [ctypes.c_char_p]
        lib.axon_stop_nrt_profile.restype = ctypes.c_int64

        @contextlib.contextmanager
        def _hook(output_dir, device_ids):
            import jax

            jax.devices()
            if device_ids:
                ids = (ctypes.c_int64 * len(device_ids))(*device_ids)
                rc = lib.axon_start_nrt_profile(ids, len(device_ids))
            else:
                rc = lib.axon_start_nrt_profile(None, 0)
            if rc != 0:
                raise RuntimeError(f"axon_start_nrt_profile rc={rc}")
            try:
                yield
            finally:
                lib.axon_stop_nrt_profile(str(output_dir).encode())

        return _hook

    mod = types.ModuleType("antenv.axon_hooks")
    _holder = {"hook": _make_hook()}
    mod.set_axon_ntff_profile_hook = lambda h: _holder.__setitem__("hook", h)
    mod.get_axon_ntff_profile_hook = lambda: _holder["hook"]
    sys.modules["antenv.axon_hooks"] = mod
    try:
        import antenv

        antenv.axon_hooks = mod
    except ImportError:
        pass


def build_nc(enable_asserts=False):
    """Build + compile the per-core Bass program (identical on all 8 cores)."""
    from concourse import bacc, mybir, tile
    from concourse.masks import make_identity

    f32 = mybir.dt.float32
    bf16 = mybir.dt.bfloat16
    AF = mybir.ActivationFunctionType
    ALU = mybir.AluOpType

    nc = bacc.Bacc(
        "TRN2",
        target_bir_lowering=False,
        debug=False,
        enable_asserts=enable_asserts,
        num_devices=N_CORES,
    )
    gT = nc.dram_tensor("gT", [N, ROWS], bf16, kind="ExternalInput").ap()
    hT = nc.dram_tensor("hT", [D, N], f32, kind="ExternalInput").ap()
    W1 = nc.dram_tensor("W1", [D, HID], f32, kind="ExternalInput").ap()
    b1 = nc.dram_tensor("b1", [HID, 1], f32, kind="ExternalInput").ap()
    W2b = nc.dram_tensor("W2b", [HID + 1, 1], f32, kind="ExternalInput").ap()
    out = nc.dram_tensor("out", [ROWS, D], f32, kind="ExternalOutput").ap()

    with tile.TileContext(nc) as tc:
        with (
            tc.tile_pool(name="const", bufs=1) as cpool,
            tc.tile_pool(name="big", bufs=1) as bigpool,
            tc.tile_pool(name="gbuf", bufs=3) as gpool,
            tc.tile_pool(name="outbuf", bufs=3) as opool,
            tc.tile_pool(name="small", bufs=2) as spool,
        ):
            W1_sb = cpool.tile([D, HID], f32)
            nc.sync.dma_start(W1_sb[:], W1[:])
            b1_sb = cpool.tile([HID, 1], f32)
            nc.sync.dma_start(b1_sb[:], b1[:])
            W2b_sb = cpool.tile([HID + 1, 1], f32)
            nc.sync.dma_start(W2b_sb[:], W2b[:])
            ident = cpool.tile([128, 128], f32)
            make_identity(nc, ident[:])
            eshift_sb = cpool.tile([128, 1], f32)
            nc.vector.memset(eshift_sb[:], ESHIFT)

            hT_sb = bigpool.tile([D, N], f32)
            nc.sync.dma_start(hT_sb[:], hT[:])

            # aT rows 0..63 hold relu(W1.T @ hT + b1); row 64 is ones so that
            # [aT; 1].T @ [W2; b2] fuses the b2 add into the score matmul.
            aT = bigpool.tile([HID + 1, N], f32)
            nc.vector.memset(aT[HID : HID + 1, :], 1.0)

            w_sb = cpool.tile([128, JC], f32)
            w_hi = cpool.tile([128, JC], bf16)
            w_rem = cpool.tile([128, JC], f32)
            w_lo = cpool.tile([128, JC], bf16)
            Hp = bigpool.tile([128, JC, NCOL], bf16)
            nc.vector.memset(Hp[:, :, 258:259], 1.0)

            with tc.tile_pool(name="ps_pre", bufs=2, space="PSUM") as ps_pre:
                for nb in range(N // 512):
                    pa = ps_pre.tile([HID, 512], f32, tag="pa")
                    nc.tensor.matmul(
                        pa[:],
                        W1_sb[:],
                        hT_sb[:, nb * 512 : (nb + 1) * 512],
                        start=True,
                        stop=True,
                    )
                    nc.scalar.activation(
                        aT[0:HID, nb * 512 : (nb + 1) * 512],
                        pa[:],
                        AF.Relu,
                        bias=b1_sb[:],
                    )

                # e laid out [128, 64]: partition = j within chunk, column = chunk.
                pe = ps_pre.tile([128, JC], f32, tag="pe")
                for c in range(JC):
                    nc.tensor.matmul(
                        pe[:, c : c + 1],
                        aT[:, c * 128 : (c + 1) * 128],
                        W2b_sb[:],
                        start=True,
                        stop=True,
                    )
                nc.scalar.activation(w_sb[:], pe[:], AF.Exp, bias=eshift_sb[:])

                # w -> bf16 hi/lo once for all chunks
                nc.vector.tensor_copy(w_hi[:], w_sb[:])
                nc.vector.tensor_tensor(
                    w_rem[:], w_sb[:], w_hi[:], op=ALU.subtract
                )
                nc.vector.tensor_copy(w_lo[:], w_rem[:])

                for jc in range(JC):
                    pt = ps_pre.tile([128, 128], f32, tag="pt")
                    nc.tensor.transpose(
                        pt[:], hT_sb[:, jc * 128 : (jc + 1) * 128], ident[:]
                    )
                    wh = spool.tile([128, 128], f32, tag="wh")
                    nc.vector.tensor_scalar_mul(
                        wh[:], pt[:], w_sb[:, jc : jc + 1]
                    )
                    nc.vector.tensor_copy(Hp[:, jc, 0:128], wh[:])
                    rem = spool.tile([128, 128], f32, tag="rem")
                    nc.vector.tensor_tensor(
                        rem[:], wh[:], Hp[:, jc, 0:128], op=ALU.subtract
                    )
                    nc.vector.tensor_copy(Hp[:, jc, 128:256], rem[:])
                    nc.vector.tensor_copy(
                        Hp[:, jc, 256:257], w_hi[:, jc : jc + 1]
                    )
                    nc.vector.tensor_copy(
                        Hp[:, jc, 257:258], w_lo[:, jc : jc + 1]
                    )

            # Main accumulation: acc[it] [128, NCOL] += G_tile.T @ H'_chunk.
            gTr = gT.rearrange("(a p) i -> p a i", p=128)  # [128, JC, ROWS]
            with tc.tile_pool(name="ps_acc", bufs=8, space="PSUM") as ps_acc:
                accs = [
                    ps_acc.tile([128, NCOL], f32, tag="acc", name=f"acc{i}")
                    for i in range(8)
                ]
                GRP = 8  # contraction chunks per DMA (2 MB transfers)
                for jg in range(JC // GRP):
                    gt = gpool.tile([128, GRP, ROWS], bf16, tag="gt")
                    nc.sync.dma_start(
                        gt[:], gTr[:, jg * GRP : (jg + 1) * GRP, :]
                    )
                    for jci in range(GRP):
                        jc = jg * GRP + jci
                        for it in range(8):
                            nc.tensor.matmul(
                                accs[it][:],
                                gt[:, jci, it * 128 : (it + 1) * 128],
                                Hp[:, jc, :],
                                start=(jc == 0),
                                stop=(jc == JC - 1),
                            )

                for it in range(8):
                    # only one PSUM operand is legal per DVE op: stage the
                    # lo-half and tail columns through SBUF first
                    tail = spool.tile([128, 3], f32, tag="tail")
                    nc.vector.tensor_copy(tail[:], accs[it][:, 256:259])
                    swe = spool.tile([128, 1], f32, tag="swe")
                    nc.vector.tensor_scalar_add(swe[:], tail[:, 0:1], 1e-30)
                    swe2 = spool.tile([128, 1], f32, tag="swe2")
                    nc.vector.tensor_tensor(
                        swe2[:], swe[:], tail[:, 1:2], op=ALU.add
                    )
                    rc = spool.tile([128, 1], f32, tag="rc")
                    nc.vector.reciprocal(rc[:], swe2[:])
                    r = spool.tile([128, 1], f32, tag="r")
                    nc.vector.tensor_mul(r[:], rc[:], tail[:, 2:3])
                    shlo = opool.tile([128, D], f32, tag="shlo")
                    nc.vector.tensor_copy(shlo[:], accs[it][:, 128:256])
                    sh = opool.tile([128, D], f32, tag="sh")
                    nc.vector.tensor_tensor(
                        sh[:], accs[it][:, 0:128], shlo[:], op=ALU.add
                    )
                    ot = opool.tile([128, D], f32, tag="ot")
                    nc.vector.tensor_scalar_mul(ot[:], sh[:], r[:])
                    nc.sync.dma_start(out[it * 128 : (it + 1) * 128, :], ot[:])

    nc.compile()
    return nc


def make_in_maps(graph_info, h, W1, b1, W2, b2):
    """Shard + lay out the full inputs for the 8 cores."""
    import ml_dtypes

    g = np.ascontiguousarray(graph_info, dtype=np.float32)
    GT = np.ascontiguousarray(g.T).astype(ml_dtypes.bfloat16)  # exact 0/1
    hT = np.ascontiguousarray(np.asarray(h, np.float32).T)     # [D, N]
    W1 = np.ascontiguousarray(np.asarray(W1, np.float32))
    b1r = np.asarray(b1, np.float32).reshape(HID, 1)
    W2b = np.concatenate(
        [np.asarray(W2, np.float32).reshape(HID, 1),
         np.asarray(b2, np.float32).reshape(1, 1)], axis=0
    )
    in_maps = []
    for c in range(N_CORES):
        in_maps.append(
            {
                "gT": GT[:, c * ROWS : (c + 1) * ROWS],
                "hT": hT,
                "W1": W1,
                "b1": b1r,
                "W2b": W2b,
            }
        )
    return in_maps


def kernel(graph_info, h, W1, b1, W2, b2):
    _install_axon_hooks_shim()
    from concourse.bass_utils import run_bass_kernel_spmd

    if "nc" not in _cache:
        _cache["nc"] = build_nc()
    nc = _cache["nc"]

    in_maps = make_in_maps(graph_info, h, W1, b1, W2, b2)
    res = run_bass_kernel_spmd(nc, in_maps, list(range(N_CORES)))
    return np.concatenate([res.results[c]["out"] for c in range(N_CORES)], axis=0)


# revision 7
# speedup vs baseline: 1.6166x; 1.3939x over previous
"""Trainium2 Bass kernel for nn_Att_mlp_softmax (GNN message passing).

Reference computation:
    e = relu(h @ W1 + b1) @ W2 + b2                       # [N, 1] per-node score
    att = softmax(where(G > 0, e.T broadcast, -9e15))     # row-wise over neighbors
    out = (G.sum(-1))[:, None] * (att @ h)                # degree-rescaled aggregation

Because the pre-softmax score of entry (i, j) depends only on column j, the
masked softmax collapses algebraically:
    att[i, j] = G[i, j] * w[j] / sum_j G[i, j] * w[j],  w = exp(e - c)
so with H' = [w * h | w | 1] (N x 130):
    Y = G @ H'
    out = Y[:, 129] * Y[:, :128] / Y[:, 128]
One big [N, N] x [N, 130] matmul replaces the N^2 softmax entirely.

Precision/perf: G is an exact 0/1 mask, so it is streamed in bf16 losslessly
(half the HBM traffic, 1 cycle/row matmul, FWL weight loads). H' is split
hi/lo into two bf16 columns per logical column (err ~2^-16), accumulated in
fp32 PSUM — near-fp32 accuracy at bf16 speed. H' chunks are built just-in-time
inside the main loop (3 fused DVE ops per chunk) so the build fully overlaps
the matmul stream.

Distribution: G is row-sharded across 8 NeuronCores (1024 rows each); h and the
MLP weights are replicated. Each core's G shard is passed pre-transposed
([N, 1024], contraction dim major) so SBUF tiles have the contraction dim on
partitions with fully contiguous DMA lines. h is passed twice: d-major (hT,
for the MLP contraction) and chunk-major natural (hc, for the H' build).
No collectives needed.

Moving-operand layout per contraction chunk jc (259 bf16 columns):
    [0:128]  hi(w * h)    [128:256] lo(w * h)
    [256]    hi(w)        [257]     lo(w)        [258] ones
"""

import numpy as np

N = 8192
D = 128
HID = 64
N_CORES = 8
ROWS = N // N_CORES          # 1024 output rows per core
JC = N // 128                # 64 contraction chunks of 128
NCOL = 259                   # moving columns per chunk (hi|lo|w_hi|w_lo|1)
ESHIFT = -4.0                # exp(e - 4): cancels exactly in the ratio, keeps
                             # w in a comfortable fp32/bf16 range

_cache = {}


def _install_axon_hooks_shim():
    """Provide antenv.axon_hooks if the image lacks it (trn_boot step 6).

    concourse.bass_utils imports it unconditionally when BASS_TRACE is set;
    without the shim that import crashes instead of degrading.
    """
    import contextlib
    import ctypes
    import sys
    import types

    try:
        import antenv.axon_hooks  # noqa: F401
        return
    except ImportError:
        pass

    so_path = "/opt/axon/libaxon_pjrt.so"

    def _make_hook():
        try:
            lib = ctypes.CDLL(so_path)
        except OSError:
            return None
        if not hasattr(lib, "axon_start_nrt_profile"):
            return None
        lib.axon_start_nrt_profile.argtypes = [
            ctypes.POINTER(ctypes.c_int64),
            ctypes.c_size_t,
        ]
        lib.axon_start_nrt_profile.restype = ctypes.c_int64
        lib.axon_stop_nrt_profile.argtypes = [ctypes.c_char_p]
        lib.axon_stop_nrt_profile.restype = ctypes.c_int64

        @contextlib.contextmanager
        def _hook(output_dir, device_ids):
            import jax

            jax.devices()
            if device_ids:
                ids = (ctypes.c_int64 * len(device_ids))(*device_ids)
                rc = lib.axon_start_nrt_profile(ids, len(device_ids))
            else:
                rc = lib.axon_start_nrt_profile(None, 0)
            if rc != 0:
                raise RuntimeError(f"axon_start_nrt_profile rc={rc}")
            try:
                yield
            finally:
                lib.axon_stop_nrt_profile(str(output_dir).encode())

        return _hook

    mod = types.ModuleType("antenv.axon_hooks")
    _holder = {"hook": _make_hook()}
    mod.set_axon_ntff_profile_hook = lambda h: _holder.__setitem__("hook", h)
    mod.get_axon_ntff_profile_hook = lambda: _holder["hook"]
    sys.modules["antenv.axon_hooks"] = mod
    try:
        import antenv

        antenv.axon_hooks = mod
    except ImportError:
        pass


def build_nc(enable_asserts=False):
    """Build + compile the per-core Bass program (identical on all 8 cores)."""
    from concourse import bacc, mybir, tile

    f32 = mybir.dt.float32
    bf16 = mybir.dt.bfloat16
    AF = mybir.ActivationFunctionType
    ALU = mybir.AluOpType

    nc = bacc.Bacc(
        "TRN2",
        target_bir_lowering=False,
        debug=False,
        enable_asserts=enable_asserts,
        num_devices=N_CORES,
    )
    gT = nc.dram_tensor("gT", [N, ROWS], bf16, kind="ExternalInput").ap()
    hT = nc.dram_tensor("hT", [D, N], f32, kind="ExternalInput").ap()
    hc = nc.dram_tensor("hc", [128, JC, D], f32, kind="ExternalInput").ap()
    W1 = nc.dram_tensor("W1", [D, HID], f32, kind="ExternalInput").ap()
    b1 = nc.dram_tensor("b1", [HID, 1], f32, kind="ExternalInput").ap()
    W2b = nc.dram_tensor("W2b", [HID + 1, 1], f32, kind="ExternalInput").ap()
    out = nc.dram_tensor("out", [ROWS, D], f32, kind="ExternalOutput").ap()

    with tile.TileContext(nc) as tc:
        with (
            tc.tile_pool(name="const", bufs=1) as cpool,
            tc.tile_pool(name="big", bufs=1) as bigpool,
            tc.tile_pool(name="gbuf", bufs=3) as gpool,
            tc.tile_pool(name="hpbuf", bufs=16) as hpool,
            tc.tile_pool(name="outbuf", bufs=3) as opool,
            tc.tile_pool(name="small", bufs=2) as spool,
        ):
            W1_sb = cpool.tile([D, HID], f32)
            nc.sync.dma_start(W1_sb[:], W1[:])
            b1_sb = cpool.tile([HID, 1], f32)
            nc.sync.dma_start(b1_sb[:], b1[:])
            W2b_sb = cpool.tile([HID + 1, 1], f32)
            nc.sync.dma_start(W2b_sb[:], W2b[:])
            eshift_sb = cpool.tile([128, 1], f32)
            nc.vector.memset(eshift_sb[:], ESHIFT)

            hT_sb = bigpool.tile([D, N], f32)
            nc.sync.dma_start(hT_sb[:], hT[:])
            hc_sb = bigpool.tile([128, JC, D], f32)
            nc.sync.dma_start(hc_sb[:], hc[:])

            # aT rows 0..63 hold relu(W1.T @ hT + b1); row 64 is ones so that
            # [aT; 1].T @ [W2; b2] fuses the b2 add into the score matmul.
            aT = bigpool.tile([HID + 1, N], f32)
            nc.vector.memset(aT[HID : HID + 1, :], 1.0)

            w_sb = cpool.tile([128, JC], f32)
            # wtail[:, :, jc] = [w_hi, w_lo, 1] for chunk jc
            wtail = cpool.tile([128, 3, JC], bf16)
            nc.vector.memset(wtail[:, 2, :], 1.0)
            w_rem = cpool.tile([128, JC], f32)

            with tc.tile_pool(name="ps_pre", bufs=2, space="PSUM") as ps_pre:
                for nb in range(N // 512):
                    pa = ps_pre.tile([HID, 512], f32, tag="pa")
                    nc.tensor.matmul(
                        pa[:],
                        W1_sb[:],
                        hT_sb[:, nb * 512 : (nb + 1) * 512],
                        start=True,
                        stop=True,
                    )
                    nc.scalar.activation(
                        aT[0:HID, nb * 512 : (nb + 1) * 512],
                        pa[:],
                        AF.Relu,
                        bias=b1_sb[:],
                    )

                # e laid out [128, 64]: partition = j within chunk, column = chunk.
                pe = ps_pre.tile([128, JC], f32, tag="pe")
                for c in range(JC):
                    nc.tensor.matmul(
                        pe[:, c : c + 1],
                        aT[:, c * 128 : (c + 1) * 128],
                        W2b_sb[:],
                        start=True,
                        stop=True,
                    )
                nc.scalar.activation(w_sb[:], pe[:], AF.Exp, bias=eshift_sb[:])

                # w -> bf16 hi/lo once for all chunks
                nc.vector.tensor_copy(wtail[:, 0, :], w_sb[:])
                nc.vector.scalar_tensor_tensor(
                    w_rem[:], w_sb[:], 1.0, wtail[:, 0, :],
                    op0=ALU.mult, op1=ALU.subtract,
                )
                nc.vector.tensor_copy(wtail[:, 1, :], w_rem[:])

            # Main accumulation: acc[it] [128, NCOL] += G_tile.T @ H'_chunk.
            gTr = gT.rearrange("(a p) i -> p a i", p=128)  # [128, JC, ROWS]
            with tc.tile_pool(name="ps_acc", bufs=8, space="PSUM") as ps_acc:
                accs = [
                    ps_acc.tile([128, NCOL], f32, tag="acc", name=f"acc{i}")
                    for i in range(8)
                ]
                GRP = 8  # contraction chunks per DMA (2 MB transfers)
                for jg in range(JC // GRP):
                    gt = gpool.tile([128, GRP, ROWS], bf16, tag="gt")
                    nc.sync.dma_start(
                        gt[:], gTr[:, jg * GRP : (jg + 1) * GRP, :]
                    )
                    for jci in range(GRP):
                        jc = jg * GRP + jci
                        # just-in-time H' chunk build: 3 DVE ops
                        hp = hpool.tile([128, NCOL], bf16, tag="hp",
                                        name=f"hp{jc}")
                        nc.vector.tensor_scalar_mul(
                            hp[:, 0:128], hc_sb[:, jc, :], w_sb[:, jc : jc + 1]
                        )
                        nc.vector.scalar_tensor_tensor(
                            hp[:, 128:256], hc_sb[:, jc, :],
                            w_sb[:, jc : jc + 1], hp[:, 0:128],
                            op0=ALU.mult, op1=ALU.subtract,
                        )
                        nc.vector.tensor_copy(hp[:, 256:259], wtail[:, :, jc])
                        for it in range(8):
                            nc.tensor.matmul(
                                accs[it][:],
                                gt[:, jci, it * 128 : (it + 1) * 128],
                                hp[:],
                                start=(jc == 0),
                                stop=(jc == JC - 1),
                            )

                for it in range(8):
                    # only one PSUM operand is legal per DVE op: stage the
                    # lo-half and tail columns through SBUF first
                    tail = spool.tile([128, 3], f32, tag="tail")
                    nc.vector.tensor_copy(tail[:], accs[it][:, 256:259])
                    swe = spool.tile([128, 1], f32, tag="swe")
                    nc.vector.tensor_scalar_add(swe[:], tail[:, 0:1], 1e-30)
                    swe2 = spool.tile([128, 1], f32, tag="swe2")
                    nc.vector.tensor_tensor(
                        swe2[:], swe[:], tail[:, 1:2], op=ALU.add
                    )
                    rc = spool.tile([128, 1], f32, tag="rc")
                    nc.vector.reciprocal(rc[:], swe2[:])
                    r = spool.tile([128, 1], f32, tag="r")
                    nc.vector.tensor_mul(r[:], rc[:], tail[:, 2:3])
                    shlo = opool.tile([128, D], f32, tag="shlo")
                    nc.vector.tensor_copy(shlo[:], accs[it][:, 128:256])
                    sh = opool.tile([128, D], f32, tag="sh")
                    nc.vector.tensor_tensor(
                        sh[:], accs[it][:, 0:128], shlo[:], op=ALU.add
                    )
                    ot = opool.tile([128, D], f32, tag="ot")
                    nc.vector.tensor_scalar_mul(ot[:], sh[:], r[:])
                    nc.sync.dma_start(out[it * 128 : (it + 1) * 128, :], ot[:])

    nc.compile()
    return nc


def make_in_maps(graph_info, h, W1, b1, W2, b2):
    """Shard + lay out the full inputs for the 8 cores."""
    import ml_dtypes

    g = np.ascontiguousarray(graph_info, dtype=np.float32)
    GT = np.ascontiguousarray(g.T).astype(ml_dtypes.bfloat16)  # exact 0/1
    h = np.asarray(h, np.float32)
    hT = np.ascontiguousarray(h.T)                             # [D, N]
    hcm = np.ascontiguousarray(
        h.reshape(JC, 128, D).transpose(1, 0, 2)               # [128, JC, D]
    )
    W1 = np.ascontiguousarray(np.asarray(W1, np.float32))
    b1r = np.asarray(b1, np.float32).reshape(HID, 1)
    W2b = np.concatenate(
        [np.asarray(W2, np.float32).reshape(HID, 1),
         np.asarray(b2, np.float32).reshape(1, 1)], axis=0
    )
    in_maps = []
    for c in range(N_CORES):
        in_maps.append(
            {
                "gT": GT[:, c * ROWS : (c + 1) * ROWS],
                "hT": hT,
                "hc": hcm,
                "W1": W1,
                "b1": b1r,
                "W2b": W2b,
            }
        )
    return in_maps


def kernel(graph_info, h, W1, b1, W2, b2):
    _install_axon_hooks_shim()
    from concourse.bass_utils import run_bass_kernel_spmd

    if "nc" not in _cache:
        _cache["nc"] = build_nc()
    nc = _cache["nc"]

    in_maps = make_in_maps(graph_info, h, W1, b1, W2, b2)
    res = run_bass_kernel_spmd(nc, in_maps, list(range(N_CORES)))
    return np.concatenate([res.results[c]["out"] for c in range(N_CORES)], axis=0)


# revision 39
# speedup vs baseline: 1.8782x; 1.1619x over previous
"""Trainium2 Bass kernel for nn_Att_mlp_softmax (GNN message passing).

Reference computation:
    e = relu(h @ W1 + b1) @ W2 + b2                       # [N, 1] per-node score
    att = softmax(where(G > 0, e.T broadcast, -9e15))     # row-wise over neighbors
    out = (G.sum(-1))[:, None] * (att @ h)                # degree-rescaled aggregation

Because the pre-softmax score of entry (i, j) depends only on column j, the
masked softmax collapses algebraically:
    att[i, j] = G[i, j] * w[j] / sum_j G[i, j] * w[j],  w = exp(e - c)
so with H' = [w * h | w | 1] (N x 130):
    Y = G @ H'
    out = Y[:, 129] * Y[:, :128] / Y[:, 128]
One big [N, N] x [N, 130] matmul replaces the N^2 softmax entirely.

Precision/perf: G is an exact 0/1 mask, so it is streamed in bf16 losslessly
(half the HBM traffic, 1 cycle/row matmul, FWL weight loads). H' is split
hi/lo into two bf16 columns per logical column (err ~2^-16), accumulated in
fp32 PSUM — near-fp32 accuracy at bf16 speed. H' chunks are built just-in-time
inside the main loop (3 fused DVE ops per chunk) so the build fully overlaps
the matmul stream.

Distribution: G is row-sharded across 8 NeuronCores (1024 rows each); h and the
MLP weights are replicated. Each core's G shard is passed pre-transposed
([N, 1024], contraction dim major) so SBUF tiles have the contraction dim on
partitions with fully contiguous DMA lines. h is passed twice: d-major bf16
hi/lo (hT2, for the MLP contraction) and chunk-major natural fp32 (hc, for the
H' build). No collectives needed.

Moving-operand layout per contraction chunk jc (259 bf16 columns):
    [0:128]  hi(w * h)    [128:256] lo(w * h)
    [256]    hi(w)        [257]     lo(w)        [258] ones
"""

import numpy as np

N = 8192
D = 128
HID = 64
N_CORES = 8
ROWS = N // N_CORES          # 1024 output rows per core
JC = N // 128                # 64 contraction chunks of 128
NCOL = 259                   # moving columns per chunk (hi|lo|w_hi|w_lo|1)
ESHIFT = -4.0                # exp(e - 4): cancels exactly in the ratio, keeps
                             # w in a comfortable fp32/bf16 range

_cache = {}


def _install_axon_hooks_shim():
    """Provide antenv.axon_hooks if the image lacks it (trn_boot step 6).

    concourse.bass_utils imports it unconditionally when BASS_TRACE is set;
    without the shim that import crashes instead of degrading.
    """
    import contextlib
    import ctypes
    import sys
    import types

    try:
        import antenv.axon_hooks  # noqa: F401
        return
    except ImportError:
        pass

    so_path = "/opt/axon/libaxon_pjrt.so"

    def _make_hook():
        try:
            lib = ctypes.CDLL(so_path)
        except OSError:
            return None
        if not hasattr(lib, "axon_start_nrt_profile"):
            return None
        lib.axon_start_nrt_profile.argtypes = [
            ctypes.POINTER(ctypes.c_int64),
            ctypes.c_size_t,
        ]
        lib.axon_start_nrt_profile.restype = ctypes.c_int64
        lib.axon_stop_nrt_profile.argtypes = [ctypes.c_char_p]
        lib.axon_stop_nrt_profile.restype = ctypes.c_int64

        @contextlib.contextmanager
        def _hook(output_dir, device_ids):
            import jax

            jax.devices()
            if device_ids:
                ids = (ctypes.c_int64 * len(device_ids))(*device_ids)
                rc = lib.axon_start_nrt_profile(ids, len(device_ids))
            else:
                rc = lib.axon_start_nrt_profile(None, 0)
            if rc != 0:
                raise RuntimeError(f"axon_start_nrt_profile rc={rc}")
            try:
                yield
            finally:
                lib.axon_stop_nrt_profile(str(output_dir).encode())

        return _hook

    mod = types.ModuleType("antenv.axon_hooks")
    _holder = {"hook": _make_hook()}
    mod.set_axon_ntff_profile_hook = lambda h: _holder.__setitem__("hook", h)
    mod.get_axon_ntff_profile_hook = lambda: _holder["hook"]
    sys.modules["antenv.axon_hooks"] = mod
    try:
        import antenv

        antenv.axon_hooks = mod
    except ImportError:
        pass


def build_nc(enable_asserts=False):
    """Build + compile the per-core Bass program (identical on all 8 cores)."""
    from concourse import bacc, mybir, tile

    f32 = mybir.dt.float32
    bf16 = mybir.dt.bfloat16
    AF = mybir.ActivationFunctionType
    ALU = mybir.AluOpType

    nc = bacc.Bacc(
        "TRN2",
        target_bir_lowering=False,
        debug=False,
        enable_asserts=enable_asserts,
        num_devices=N_CORES,
    )
    gT = nc.dram_tensor("gT", [N, ROWS], bf16, kind="ExternalInput").ap()
    # hT split hi/lo in bf16 (host-side) so the MLP matmuls run at bf16 speed;
    # packed as [D, 2, N] so one DMA covers both halves
    hT2 = nc.dram_tensor("hT2", [D, 2, N], bf16, kind="ExternalInput").ap()
    hc = nc.dram_tensor("hc", [128, JC, D], f32, kind="ExternalInput").ap()
    W1h = nc.dram_tensor("W1h", [D, HID], bf16, kind="ExternalInput").ap()
    W1l = nc.dram_tensor("W1l", [D, HID], bf16, kind="ExternalInput").ap()
    b1 = nc.dram_tensor("b1", [HID, 1], f32, kind="ExternalInput").ap()
    # W2 in bf16 column pairs: W2p1 = [W2_hi | W2_lo], W2p2 = [W2_hi | 0].
    # e accumulates aThi @ W2p1 + aTlo @ W2p2 column-pair-wise so the three
    # hi/lo cross terms land in two psum columns summed afterwards.
    W2p1 = nc.dram_tensor("W2p1", [HID, 2], bf16, kind="ExternalInput").ap()
    W2p2 = nc.dram_tensor("W2p2", [HID, 2], bf16, kind="ExternalInput").ap()
    b2 = nc.dram_tensor("b2", [1, 1], f32, kind="ExternalInput").ap()
    out = nc.dram_tensor("out", [ROWS, D], f32, kind="ExternalOutput").ap()

    with tile.TileContext(nc) as tc:
        with (
            tc.tile_pool(name="const", bufs=1) as cpool,
            tc.tile_pool(name="big", bufs=1) as bigpool,
            tc.tile_pool(name="gbuf", bufs=3) as gpool,
            tc.tile_pool(name="hpbuf", bufs=16) as hpool,
            tc.tile_pool(name="outbuf", bufs=3) as opool,
            tc.tile_pool(name="small", bufs=2) as spool,
        ):
            W1h_sb = cpool.tile([D, HID], bf16)
            nc.sync.dma_start(W1h_sb[:], W1h[:])
            W1l_sb = cpool.tile([D, HID], bf16)
            nc.sync.dma_start(W1l_sb[:], W1l[:])
            b1_sb = cpool.tile([HID, 1], f32)
            nc.sync.dma_start(b1_sb[:], b1[:])
            W2p1_sb = cpool.tile([HID, 2], bf16)
            nc.sync.dma_start(W2p1_sb[:], W2p1[:])
            W2p2_sb = cpool.tile([HID, 2], bf16)
            nc.sync.dma_start(W2p2_sb[:], W2p2[:])
            b2_sb = cpool.tile([1, 1], f32)
            nc.sync.dma_start(b2_sb[:], b2[:])
            ones_row = cpool.tile([1, 128], f32)
            nc.vector.memset(ones_row[:], 1.0)

            # h DMAs go on the gpsimd (SWDGE) queue so their dispatch doesn't
            # serialize with the G stream on the sync (HWDGE) queue. Both h
            # tensors are chunked and interleaved by need-time: hT2 pieces
            # feed the MLP from ~10us, hc pieces feed the H' builds from
            # ~36us (a monolithic hc was measured landing at ~41us because
            # the G stream steals SDMA round-robin share, stalling hp0).
            NHCH = 8
            hT2_sb = bigpool.tile([D, 2, N], bf16)
            hc_sb = bigpool.tile([128, JC, D], f32)

            def _hT2_piece(q):
                sl = slice(q * (N // NHCH), (q + 1) * (N // NHCH))
                nc.gpsimd.dma_start(hT2_sb[:, :, sl], hT2[:, :, sl])

            def _hc_piece(p):
                nc.gpsimd.dma_start(
                    hc_sb[:, p * 8 : (p + 1) * 8, :],
                    hc[:, p * 8 : (p + 1) * 8, :],
                )

            _hT2_piece(0)
            _hT2_piece(1)
            _hT2_piece(2)
            for p in range(5):
                _hc_piece(p)
                _hT2_piece(3 + p)
            _hc_piece(5)
            _hc_piece(6)
            _hc_piece(7)

            aTh = bigpool.tile([HID, N], bf16)
            aTl = bigpool.tile([HID, N], bf16)

            w_sb = cpool.tile([128, JC], f32)
            # wtail[:, :, jc] = [w_hi, w_lo, 1] for chunk jc
            wtail = cpool.tile([128, 3, JC], bf16)
            nc.vector.memset(wtail[:, 2, :], 1.0)
            w_rem = cpool.tile([128, JC], f32)

            with tc.tile_pool(name="ps_pre", bufs=2, space="PSUM") as ps_pre:
                # ~4us of dummy matmuls on a zero tile: trips the PE HAM
                # activity monitor out of its 1.2 GHz cold state before the
                # real MLP arrives (no input deps, runs during the h DMA)
                warm = cpool.tile([128, 512], bf16)
                nc.vector.memset(warm[:], 0.0)
                pwarm = ps_pre.tile([128, 512], f32, tag="pwarm")
                for _ in range(32):
                    nc.tensor.matmul(
                        pwarm[:], warm[:, 0:128], warm[:], start=True, stop=True
                    )

                # ebias = b2 + ESHIFT broadcast to 128 partitions via a K=1
                # matmul (avoids any slow single-partition ops)
                pb2 = ps_pre.tile([128, 1], f32, tag="pb2")
                nc.tensor.matmul(pb2[:], ones_row[:], b2_sb[:], start=True,
                                 stop=True)
                ebias_sb = cpool.tile([128, 1], f32)
                nc.vector.tensor_scalar_add(ebias_sb[:], pb2[:], ESHIFT)

                af_all = bigpool.tile([HID, N], f32)
                for nb in range(N // 512):
                    # a = h @ W1 in 3 bf16 terms: hi*hi + hi*lo + lo*hi
                    # (the dropped lo*lo term is ~2^-32 relative)
                    pa = ps_pre.tile([HID, 512], f32, tag="pa")
                    sl = slice(nb * 512, (nb + 1) * 512)
                    nc.tensor.matmul(
                        pa[:], W1h_sb[:], hT2_sb[:, 0, sl], start=True,
                        stop=False,
                    )
                    nc.tensor.matmul(
                        pa[:], W1l_sb[:], hT2_sb[:, 0, sl], start=False,
                        stop=False,
                    )
                    nc.tensor.matmul(
                        pa[:], W1h_sb[:], hT2_sb[:, 1, sl], start=False,
                        stop=True,
                    )
                    nc.scalar.activation(
                        af_all[:, sl], pa[:], AF.Relu, bias=b1_sb[:]
                    )
                    # a -> bf16 hi/lo (keeps the score matmul on the FWL
                    # path), batched per 4 blocks to amortize DVE op overhead
                    if nb % 4 == 3:
                        bl = slice((nb - 3) * 512, (nb + 1) * 512)
                        nc.vector.tensor_copy(aTh[:, bl], af_all[:, bl])
                        nc.vector.scalar_tensor_tensor(
                            aTl[:, bl], af_all[:, bl], 1.0, aTh[:, bl],
                            op0=ALU.mult, op1=ALU.subtract,
                        )

                # e laid out [128, 64]: partition = j within chunk, column =
                # chunk. Two bf16 matmuls per chunk into a column pair that is
                # summed after: e = aThi@W2hi + aThi@W2lo + aTlo@W2hi.
                pe2 = ps_pre.tile([128, JC, 2], f32, tag="pe2")
                pe2s = cpool.tile([128, JC, 2], f32)
                pe_sum = cpool.tile([128, JC], f32)
                # combine -> exp -> w hi/lo per quarter so the first H' chunks
                # unblock the main loop while later scores still compute
                QW = JC // 4
                for q in range(4):
                    for c in range(q * QW, (q + 1) * QW):
                        nc.tensor.matmul(
                            pe2[:, c, :],
                            aTh[:, c * 128 : (c + 1) * 128],
                            W2p1_sb[:],
                            start=True,
                            stop=False,
                        )
                        nc.tensor.matmul(
                            pe2[:, c, :],
                            aTl[:, c * 128 : (c + 1) * 128],
                            W2p2_sb[:],
                            start=False,
                            stop=True,
                        )
                    ql = slice(q * QW, (q + 1) * QW)
                    nc.vector.tensor_copy(pe2s[:, ql, :], pe2[:, ql, :])
                    nc.vector.tensor_tensor(
                        pe_sum[:, ql], pe2s[:, ql, 0], pe2s[:, ql, 1],
                        op=ALU.add,
                    )
                    nc.scalar.activation(
                        w_sb[:, ql], pe_sum[:, ql], AF.Exp, bias=ebias_sb[:]
                    )
                    nc.vector.tensor_copy(wtail[:, 0, ql], w_sb[:, ql])
                    nc.vector.scalar_tensor_tensor(
                        w_rem[:, ql], w_sb[:, ql], 1.0, wtail[:, 0, ql],
                        op0=ALU.mult, op1=ALU.subtract,
                    )
                    nc.vector.tensor_copy(wtail[:, 1, ql], w_rem[:, ql])

            # Main accumulation: acc[it] [128, NCOL] += G_tile.T @ H'_chunk.
            gTr = gT.rearrange("(a p) i -> p a i", p=128)  # [128, JC, ROWS]
            with tc.tile_pool(name="ps_acc", bufs=8, space="PSUM") as ps_acc:
                accs = [
                    ps_acc.tile([128, NCOL], f32, tag="acc", name=f"acc{i}")
                    for i in range(8)
                ]
                GRP = 8  # contraction chunks per DMA (2 MB transfers)
                for jg in range(JC // GRP):
                    gt = gpool.tile([128, GRP, ROWS], bf16, tag="gt")
                    nc.sync.dma_start(
                        gt[:], gTr[:, jg * GRP : (jg + 1) * GRP, :]
                    )
                    for jci in range(GRP):
                        jc = jg * GRP + jci
                        # just-in-time H' chunk build: 3 DVE ops
                        hp = hpool.tile([128, NCOL], bf16, tag="hp",
                                        name=f"hp{jc}")
                        nc.vector.tensor_scalar_mul(
                            hp[:, 0:128], hc_sb[:, jc, :], w_sb[:, jc : jc + 1]
                        )
                        nc.vector.scalar_tensor_tensor(
                            hp[:, 128:256], hc_sb[:, jc, :],
                            w_sb[:, jc : jc + 1], hp[:, 0:128],
                            op0=ALU.mult, op1=ALU.subtract,
                        )
                        nc.vector.tensor_copy(hp[:, 256:259], wtail[:, :, jc])
                        for it in range(8):
                            nc.tensor.matmul(
                                accs[it][:],
                                gt[:, jci, it * 128 : (it + 1) * 128],
                                hp[:],
                                start=(jc == 0),
                                stop=(jc == JC - 1),
                            )

                # epilogue: r = num / (sw + eps) batched across all 8 banks,
                # then 3 [128, 128] DVE ops + 1 DMA per bank.
                # (only one PSUM operand is legal per DVE op)
                # [128, it, (sw_hi, sw_lo, num)]: writes stay contiguous in the
                # innermost dim (non-contiguous DVE writes mis-lower on HW)
                tails = spool.tile([128, 8, 3], f32, tag="tails")
                for it in range(8):
                    nc.vector.tensor_copy(tails[:, it, :], accs[it][:, 256:259])
                swsum = spool.tile([128, 8], f32, tag="swsum")
                nc.vector.tensor_tensor(
                    swsum[:], tails[:, :, 0], tails[:, :, 1], op=ALU.add
                )
                swe = spool.tile([128, 8], f32, tag="swe")
                nc.vector.tensor_scalar_add(swe[:], swsum[:], 1e-30)
                rc8 = spool.tile([128, 8], f32, tag="rc8")
                nc.vector.reciprocal(rc8[:], swe[:])
                r8 = spool.tile([128, 8], f32, tag="r8")
                nc.vector.tensor_mul(r8[:], rc8[:], tails[:, :, 2])
                ot_all = opool.tile([128, 8, D], f32, tag="ot_all", bufs=1)
                for it in range(8):
                    shlo = opool.tile([128, D], f32, tag="shlo")
                    nc.vector.tensor_copy(shlo[:], accs[it][:, 128:256])
                    sh = opool.tile([128, D], f32, tag="sh")
                    nc.vector.tensor_tensor(
                        sh[:], accs[it][:, 0:128], shlo[:], op=ALU.add
                    )
                    nc.vector.tensor_scalar_mul(
                        ot_all[:, it, :], sh[:], r8[:, it : it + 1]
                    )
                nc.sync.dma_start(
                    out.rearrange("(a p) d -> p a d", p=128), ot_all[:]
                )

    nc.compile()
    return nc


def make_in_maps(graph_info, h, W1, b1, W2, b2):
    """Shard + lay out the full inputs for the 8 cores."""
    import ml_dtypes

    bf16 = ml_dtypes.bfloat16

    def hilo(x):
        hi = x.astype(bf16)
        lo = (x - hi.astype(np.float32)).astype(bf16)
        return hi, lo

    g = np.ascontiguousarray(graph_info, dtype=np.float32)
    GT = np.ascontiguousarray(g.T).astype(bf16)                # exact 0/1
    h = np.asarray(h, np.float32)
    hT = np.ascontiguousarray(h.T)                             # [D, N]
    hTh, hTl = hilo(hT)
    hT2 = np.ascontiguousarray(np.stack([hTh, hTl], axis=1))   # [D, 2, N]
    hcm = np.ascontiguousarray(
        h.reshape(JC, 128, D).transpose(1, 0, 2)               # [128, JC, D]
    )
    W1h, W1l = hilo(np.ascontiguousarray(np.asarray(W1, np.float32)))
    b1r = np.asarray(b1, np.float32).reshape(HID, 1)
    W2h, W2l = hilo(np.asarray(W2, np.float32).reshape(HID, 1))
    W2p1 = np.ascontiguousarray(np.concatenate([W2h, W2l], axis=1))
    W2p2 = np.ascontiguousarray(
        np.concatenate([W2h, np.zeros_like(W2h)], axis=1)
    )
    b2r = np.asarray(b2, np.float32).reshape(1, 1)
    in_maps = []
    for c in range(N_CORES):
        in_maps.append(
            {
                "gT": GT[:, c * ROWS : (c + 1) * ROWS],
                "hT2": hT2,
                "hc": hcm,
                "W1h": W1h,
                "W1l": W1l,
                "b1": b1r,
                "W2p1": W2p1,
                "W2p2": W2p2,
                "b2": b2r,
            }
        )
    return in_maps


def kernel(graph_info, h, W1, b1, W2, b2):
    _install_axon_hooks_shim()
    from concourse.bass_utils import run_bass_kernel_spmd

    if "nc" not in _cache:
        _cache["nc"] = build_nc()
    nc = _cache["nc"]

    in_maps = make_in_maps(graph_info, h, W1, b1, W2, b2)
    res = run_bass_kernel_spmd(nc, in_maps, list(range(N_CORES)))
    return np.concatenate([res.results[c]["out"] for c in range(N_CORES)], axis=0)


# revision 40
# speedup vs baseline: 1.9790x; 1.0536x over previous
"""Trainium2 Bass kernel for nn_Att_mlp_softmax (GNN message passing).

Reference computation:
    e = relu(h @ W1 + b1) @ W2 + b2                       # [N, 1] per-node score
    att = softmax(where(G > 0, e.T broadcast, -9e15))     # row-wise over neighbors
    out = (G.sum(-1))[:, None] * (att @ h)                # degree-rescaled aggregation

Because the pre-softmax score of entry (i, j) depends only on column j, the
masked softmax collapses algebraically:
    att[i, j] = G[i, j] * w[j] / sum_j G[i, j] * w[j],  w = exp(e - c)
so with H' = [w * h | w | 1] (N x 130):
    Y = G @ H'
    out = Y[:, 129] * Y[:, :128] / Y[:, 128]
One big [N, N] x [N, 130] matmul replaces the N^2 softmax entirely.

Precision/perf: G is an exact 0/1 mask, so it is streamed in bf16 losslessly
(half the HBM traffic, 1 cycle/row matmul, FWL weight loads). H' is split
hi/lo into two bf16 columns per logical column (err ~2^-16), accumulated in
fp32 PSUM — near-fp32 accuracy at bf16 speed. H' chunks are built just-in-time
inside the main loop (3 fused DVE ops per chunk) so the build fully overlaps
the matmul stream.

Distribution: G is row-sharded across 8 NeuronCores (1024 rows each); h and the
MLP weights are replicated. Each core's G shard is passed pre-transposed
([N, 1024], contraction dim major) so SBUF tiles have the contraction dim on
partitions with fully contiguous DMA lines. h is passed twice: d-major bf16
hi/lo (hT2, for the MLP contraction) and chunk-major natural fp32 (hc, for the
H' build). No collectives needed.

Moving-operand layout per contraction chunk jc (259 bf16 columns):
    [0:128]  hi(w * h)    [128:256] lo(w * h)
    [256]    hi(w)        [257]     lo(w)        [258] ones
"""

import numpy as np

N = 8192
D = 128
HID = 64
N_CORES = 8
ROWS = N // N_CORES          # 1024 output rows per core
JC = N // 128                # 64 contraction chunks of 128
NCOL = 259                   # moving columns per chunk (hi|lo|w_hi|w_lo|1)
ESHIFT = -4.0                # exp(e - 4): cancels exactly in the ratio, keeps
                             # w in a comfortable fp32/bf16 range

_cache = {}


def _install_axon_hooks_shim():
    """Provide antenv.axon_hooks if the image lacks it (trn_boot step 6).

    concourse.bass_utils imports it unconditionally when BASS_TRACE is set;
    without the shim that import crashes instead of degrading.
    """
    import contextlib
    import ctypes
    import sys
    import types

    try:
        import antenv.axon_hooks  # noqa: F401
        return
    except ImportError:
        pass

    so_path = "/opt/axon/libaxon_pjrt.so"

    def _make_hook():
        try:
            lib = ctypes.CDLL(so_path)
        except OSError:
            return None
        if not hasattr(lib, "axon_start_nrt_profile"):
            return None
        lib.axon_start_nrt_profile.argtypes = [
            ctypes.POINTER(ctypes.c_int64),
            ctypes.c_size_t,
        ]
        lib.axon_start_nrt_profile.restype = ctypes.c_int64
        lib.axon_stop_nrt_profile.argtypes = [ctypes.c_char_p]
        lib.axon_stop_nrt_profile.restype = ctypes.c_int64

        @contextlib.contextmanager
        def _hook(output_dir, device_ids):
            import jax

            jax.devices()
            if device_ids:
                ids = (ctypes.c_int64 * len(device_ids))(*device_ids)
                rc = lib.axon_start_nrt_profile(ids, len(device_ids))
            else:
                rc = lib.axon_start_nrt_profile(None, 0)
            if rc != 0:
                raise RuntimeError(f"axon_start_nrt_profile rc={rc}")
            try:
                yield
            finally:
                lib.axon_stop_nrt_profile(str(output_dir).encode())

        return _hook

    mod = types.ModuleType("antenv.axon_hooks")
    _holder = {"hook": _make_hook()}
    mod.set_axon_ntff_profile_hook = lambda h: _holder.__setitem__("hook", h)
    mod.get_axon_ntff_profile_hook = lambda: _holder["hook"]
    sys.modules["antenv.axon_hooks"] = mod
    try:
        import antenv

        antenv.axon_hooks = mod
    except ImportError:
        pass


def build_nc(enable_asserts=False):
    """Build + compile the per-core Bass program (identical on all 8 cores)."""
    from concourse import bacc, mybir, tile

    f32 = mybir.dt.float32
    bf16 = mybir.dt.bfloat16
    AF = mybir.ActivationFunctionType
    ALU = mybir.AluOpType

    nc = bacc.Bacc(
        "TRN2",
        target_bir_lowering=False,
        debug=False,
        enable_asserts=enable_asserts,
        num_devices=N_CORES,
    )
    gT = nc.dram_tensor("gT", [N, ROWS], bf16, kind="ExternalInput").ap()
    # hT split hi/lo in bf16 (host-side) so the MLP matmuls run at bf16 speed;
    # packed as [D, 2, N] so one DMA covers both halves
    hT2 = nc.dram_tensor("hT2", [D, 2, N], bf16, kind="ExternalInput").ap()
    hc = nc.dram_tensor("hc", [128, JC, D], f32, kind="ExternalInput").ap()
    W1h = nc.dram_tensor("W1h", [D, HID], bf16, kind="ExternalInput").ap()
    W1l = nc.dram_tensor("W1l", [D, HID], bf16, kind="ExternalInput").ap()
    b1 = nc.dram_tensor("b1", [HID, 1], f32, kind="ExternalInput").ap()
    # W2 in bf16 column pairs: W2p1 = [W2_hi | W2_lo], W2p2 = [W2_hi | 0].
    # e accumulates aThi @ W2p1 + aTlo @ W2p2 column-pair-wise so the three
    # hi/lo cross terms land in two psum columns summed afterwards.
    W2p1 = nc.dram_tensor("W2p1", [HID, 2], bf16, kind="ExternalInput").ap()
    W2p2 = nc.dram_tensor("W2p2", [HID, 2], bf16, kind="ExternalInput").ap()
    b2 = nc.dram_tensor("b2", [1, 1], f32, kind="ExternalInput").ap()
    out = nc.dram_tensor("out", [ROWS, D], f32, kind="ExternalOutput").ap()

    with tile.TileContext(nc) as tc:
        with (
            tc.tile_pool(name="const", bufs=1) as cpool,
            tc.tile_pool(name="big", bufs=1) as bigpool,
            tc.tile_pool(name="gbuf", bufs=3) as gpool,
            tc.tile_pool(name="hpbuf", bufs=16) as hpool,
            tc.tile_pool(name="outbuf", bufs=3) as opool,
            tc.tile_pool(name="small", bufs=2) as spool,
        ):
            W1h_sb = cpool.tile([D, HID], bf16)
            nc.sync.dma_start(W1h_sb[:], W1h[:])
            W1l_sb = cpool.tile([D, HID], bf16)
            nc.sync.dma_start(W1l_sb[:], W1l[:])
            b1_sb = cpool.tile([HID, 1], f32)
            nc.sync.dma_start(b1_sb[:], b1[:])
            W2p1_sb = cpool.tile([HID, 2], bf16)
            nc.sync.dma_start(W2p1_sb[:], W2p1[:])
            W2p2_sb = cpool.tile([HID, 2], bf16)
            nc.sync.dma_start(W2p2_sb[:], W2p2[:])
            b2_sb = cpool.tile([1, 1], f32)
            nc.sync.dma_start(b2_sb[:], b2[:])
            ones_row = cpool.tile([1, 128], f32)
            nc.vector.memset(ones_row[:], 1.0)

            # h DMAs go on the gpsimd (SWDGE) queue so their dispatch doesn't
            # serialize with the G stream on the sync (HWDGE) queue; the MLP
            # h input is chunked so compute starts after the first quarter.
            NHCH = 8
            hT2_sb = bigpool.tile([D, 2, N], bf16)
            for q in range(NHCH):
                sl = slice(q * (N // NHCH), (q + 1) * (N // NHCH))
                nc.gpsimd.dma_start(hT2_sb[:, :, sl], hT2[:, :, sl])
            hc_sb = bigpool.tile([128, JC, D], f32)
            nc.gpsimd.dma_start(hc_sb[:], hc[:])

            aTh = bigpool.tile([HID, N], bf16)
            aTl = bigpool.tile([HID, N], bf16)

            w_sb = cpool.tile([128, JC], f32)
            # wtail[:, :, jc] = [w_hi, w_lo, 1] for chunk jc
            wtail = cpool.tile([128, 3, JC], bf16)
            nc.vector.memset(wtail[:, 2, :], 1.0)
            w_rem = cpool.tile([128, JC], f32)

            with tc.tile_pool(name="ps_pre", bufs=2, space="PSUM") as ps_pre:
                # ~4us of dummy matmuls on a zero tile: trips the PE HAM
                # activity monitor out of its 1.2 GHz cold state before the
                # real MLP arrives (no input deps, runs during the h DMA)
                warm = cpool.tile([128, 512], bf16)
                nc.vector.memset(warm[:], 0.0)
                pwarm = ps_pre.tile([128, 512], f32, tag="pwarm")
                for _ in range(32):
                    nc.tensor.matmul(
                        pwarm[:], warm[:, 0:128], warm[:], start=True, stop=True
                    )

                # ebias = b2 + ESHIFT broadcast to 128 partitions via a K=1
                # matmul (avoids any slow single-partition ops)
                pb2 = ps_pre.tile([128, 1], f32, tag="pb2")
                nc.tensor.matmul(pb2[:], ones_row[:], b2_sb[:], start=True,
                                 stop=True)
                ebias_sb = cpool.tile([128, 1], f32)
                nc.vector.tensor_scalar_add(ebias_sb[:], pb2[:], ESHIFT)

                af_all = bigpool.tile([HID, N], f32)
                for nb in range(N // 512):
                    # a = h @ W1 in 3 bf16 terms: hi*hi + hi*lo + lo*hi
                    # (the dropped lo*lo term is ~2^-32 relative)
                    pa = ps_pre.tile([HID, 512], f32, tag="pa")
                    sl = slice(nb * 512, (nb + 1) * 512)
                    nc.tensor.matmul(
                        pa[:], W1h_sb[:], hT2_sb[:, 0, sl], start=True,
                        stop=False,
                    )
                    nc.tensor.matmul(
                        pa[:], W1l_sb[:], hT2_sb[:, 0, sl], start=False,
                        stop=False,
                    )
                    nc.tensor.matmul(
                        pa[:], W1h_sb[:], hT2_sb[:, 1, sl], start=False,
                        stop=True,
                    )
                    nc.scalar.activation(
                        af_all[:, sl], pa[:], AF.Relu, bias=b1_sb[:]
                    )
                    # a -> bf16 hi/lo (keeps the score matmul on the FWL
                    # path), batched per 4 blocks to amortize DVE op overhead
                    if nb % 4 == 3:
                        bl = slice((nb - 3) * 512, (nb + 1) * 512)
                        nc.vector.tensor_copy(aTh[:, bl], af_all[:, bl])
                        nc.vector.scalar_tensor_tensor(
                            aTl[:, bl], af_all[:, bl], 1.0, aTh[:, bl],
                            op0=ALU.mult, op1=ALU.subtract,
                        )

                # e laid out [128, 64]: partition = j within chunk, column =
                # chunk. Two bf16 matmuls per chunk into a column pair that is
                # summed after: e = aThi@W2hi + aThi@W2lo + aTlo@W2hi.
                pe2 = ps_pre.tile([128, JC, 2], f32, tag="pe2")
                pe2s = cpool.tile([128, JC, 2], f32)
                pe_sum = cpool.tile([128, JC], f32)
                # combine -> exp -> w hi/lo per quarter so the first H' chunks
                # unblock the main loop while later scores still compute
                QW = JC // 4
                for q in range(4):
                    for c in range(q * QW, (q + 1) * QW):
                        nc.tensor.matmul(
                            pe2[:, c, :],
                            aTh[:, c * 128 : (c + 1) * 128],
                            W2p1_sb[:],
                            start=True,
                            stop=False,
                        )
                        nc.tensor.matmul(
                            pe2[:, c, :],
                            aTl[:, c * 128 : (c + 1) * 128],
                            W2p2_sb[:],
                            start=False,
                            stop=True,
                        )
                    ql = slice(q * QW, (q + 1) * QW)
                    nc.vector.tensor_copy(pe2s[:, ql, :], pe2[:, ql, :])
                    nc.vector.tensor_tensor(
                        pe_sum[:, ql], pe2s[:, ql, 0], pe2s[:, ql, 1],
                        op=ALU.add,
                    )
                    nc.scalar.activation(
                        w_sb[:, ql], pe_sum[:, ql], AF.Exp, bias=ebias_sb[:]
                    )
                    nc.vector.tensor_copy(wtail[:, 0, ql], w_sb[:, ql])
                    nc.vector.scalar_tensor_tensor(
                        w_rem[:, ql], w_sb[:, ql], 1.0, wtail[:, 0, ql],
                        op0=ALU.mult, op1=ALU.subtract,
                    )
                    nc.vector.tensor_copy(wtail[:, 1, ql], w_rem[:, ql])

            # Main accumulation: acc[it] [128, NCOL] += G_tile.T @ H'_chunk.
            gTr = gT.rearrange("(a p) i -> p a i", p=128)  # [128, JC, ROWS]
            with tc.tile_pool(name="ps_acc", bufs=8, space="PSUM") as ps_acc:
                accs = [
                    ps_acc.tile([128, NCOL], f32, tag="acc", name=f"acc{i}")
                    for i in range(8)
                ]
                GRP = 8  # contraction chunks per DMA (2 MB transfers)
                for jg in range(JC // GRP):
                    gt = gpool.tile([128, GRP, ROWS], bf16, tag="gt")
                    nc.sync.dma_start(
                        gt[:], gTr[:, jg * GRP : (jg + 1) * GRP, :]
                    )
                    for jci in range(GRP):
                        jc = jg * GRP + jci
                        # just-in-time H' chunk build: 3 DVE ops
                        hp = hpool.tile([128, NCOL], bf16, tag="hp",
                                        name=f"hp{jc}")
                        nc.vector.tensor_scalar_mul(
                            hp[:, 0:128], hc_sb[:, jc, :], w_sb[:, jc : jc + 1]
                        )
                        nc.vector.scalar_tensor_tensor(
                            hp[:, 128:256], hc_sb[:, jc, :],
                            w_sb[:, jc : jc + 1], hp[:, 0:128],
                            op0=ALU.mult, op1=ALU.subtract,
                        )
                        nc.vector.tensor_copy(hp[:, 256:259], wtail[:, :, jc])
                        for it in range(8):
                            nc.tensor.matmul(
                                accs[it][:],
                                gt[:, jci, it * 128 : (it + 1) * 128],
                                hp[:],
                                start=(jc == 0),
                                stop=(jc == JC - 1),
                            )

                # epilogue: r = num / (sw + eps) batched across all 8 banks,
                # then 3 [128, 128] DVE ops + 1 DMA per bank.
                # (only one PSUM operand is legal per DVE op)
                # [128, it, (sw_hi, sw_lo, num)]: writes stay contiguous in the
                # innermost dim (non-contiguous DVE writes mis-lower on HW)
                tails = spool.tile([128, 8, 3], f32, tag="tails")
                for it in range(8):
                    nc.vector.tensor_copy(tails[:, it, :], accs[it][:, 256:259])
                swsum = spool.tile([128, 8], f32, tag="swsum")
                nc.vector.tensor_tensor(
                    swsum[:], tails[:, :, 0], tails[:, :, 1], op=ALU.add
                )
                swe = spool.tile([128, 8], f32, tag="swe")
                nc.vector.tensor_scalar_add(swe[:], swsum[:], 1e-30)
                rc8 = spool.tile([128, 8], f32, tag="rc8")
                nc.vector.reciprocal(rc8[:], swe[:])
                r8 = spool.tile([128, 8], f32, tag="r8")
                nc.vector.tensor_mul(r8[:], rc8[:], tails[:, :, 2])
                ot_all = opool.tile([128, 8, D], f32, tag="ot_all", bufs=1)
                for it in range(8):
                    shlo = opool.tile([128, D], f32, tag="shlo")
                    nc.vector.tensor_copy(shlo[:], accs[it][:, 128:256])
                    sh = opool.tile([128, D], f32, tag="sh")
                    nc.vector.tensor_tensor(
                        sh[:], accs[it][:, 0:128], shlo[:], op=ALU.add
                    )
                    nc.vector.tensor_scalar_mul(
                        ot_all[:, it, :], sh[:], r8[:, it : it + 1]
                    )
                nc.sync.dma_start(
                    out.rearrange("(a p) d -> p a d", p=128), ot_all[:]
                )

    nc.compile()
    return nc


def make_in_maps(graph_info, h, W1, b1, W2, b2):
    """Shard + lay out the full inputs for the 8 cores."""
    import ml_dtypes

    bf16 = ml_dtypes.bfloat16

    def hilo(x):
        hi = x.astype(bf16)
        lo = (x - hi.astype(np.float32)).astype(bf16)
        return hi, lo

    g = np.ascontiguousarray(graph_info, dtype=np.float32)
    GT = np.ascontiguousarray(g.T).astype(bf16)                # exact 0/1
    h = np.asarray(h, np.float32)
    hT = np.ascontiguousarray(h.T)                             # [D, N]
    hTh, hTl = hilo(hT)
    hT2 = np.ascontiguousarray(np.stack([hTh, hTl], axis=1))   # [D, 2, N]
    hcm = np.ascontiguousarray(
        h.reshape(JC, 128, D).transpose(1, 0, 2)               # [128, JC, D]
    )
    W1h, W1l = hilo(np.ascontiguousarray(np.asarray(W1, np.float32)))
    b1r = np.asarray(b1, np.float32).reshape(HID, 1)
    W2h, W2l = hilo(np.asarray(W2, np.float32).reshape(HID, 1))
    W2p1 = np.ascontiguousarray(np.concatenate([W2h, W2l], axis=1))
    W2p2 = np.ascontiguousarray(
        np.concatenate([W2h, np.zeros_like(W2h)], axis=1)
    )
    b2r = np.asarray(b2, np.float32).reshape(1, 1)
    in_maps = []
    for c in range(N_CORES):
        in_maps.append(
            {
                "gT": GT[:, c * ROWS : (c + 1) * ROWS],
                "hT2": hT2,
                "hc": hcm,
                "W1h": W1h,
                "W1l": W1l,
                "b1": b1r,
                "W2p1": W2p1,
                "W2p2": W2p2,
                "b2": b2r,
            }
        )
    return in_maps


def kernel(graph_info, h, W1, b1, W2, b2):
    _install_axon_hooks_shim()
    from concourse.bass_utils import run_bass_kernel_spmd

    if "nc" not in _cache:
        _cache["nc"] = build_nc()
    nc = _cache["nc"]

    in_maps = make_in_maps(graph_info, h, W1, b1, W2, b2)
    res = run_bass_kernel_spmd(nc, in_maps, list(range(N_CORES)))
    return np.concatenate([res.results[c]["out"] for c in range(N_CORES)], axis=0)


# revision 47
# speedup vs baseline: 2.0243x; 1.0229x over previous
"""Trainium2 Bass kernel for nn_Att_mlp_softmax (GNN message passing).

Reference computation:
    e = relu(h @ W1 + b1) @ W2 + b2                       # [N, 1] per-node score
    att = softmax(where(G > 0, e.T broadcast, -9e15))     # row-wise over neighbors
    out = (G.sum(-1))[:, None] * (att @ h)                # degree-rescaled aggregation

Because the pre-softmax score of entry (i, j) depends only on column j, the
masked softmax collapses algebraically:
    att[i, j] = G[i, j] * w[j] / sum_j G[i, j] * w[j],  w = exp(e - c)
so with H' = [w * h | w | 1] (N x 130):
    Y = G @ H'
    out = Y[:, 129] * Y[:, :128] / Y[:, 128]
One big [N, N] x [N, 130] matmul replaces the N^2 softmax entirely.

Precision/perf: G is an exact 0/1 mask, so it is streamed in bf16 losslessly
(half the HBM traffic, 1 cycle/row matmul, FWL weight loads). H' is split
hi/lo into two bf16 columns per logical column (err ~2^-16), accumulated in
fp32 PSUM — near-fp32 accuracy at bf16 speed. H' chunks are built just-in-time
inside the main loop (3 fused DVE ops per chunk) so the build fully overlaps
the matmul stream.

Distribution: G is row-sharded across 8 NeuronCores (1024 rows each); h and the
MLP weights are replicated. Each core's G shard is passed pre-transposed
([N, 1024], contraction dim major) so SBUF tiles have the contraction dim on
partitions with fully contiguous DMA lines. h is passed twice: d-major bf16
hi/lo (hT2, for the MLP contraction) and chunk-major natural fp32 (hc, for the
H' build). No collectives needed.

Moving-operand layout per contraction chunk jc (259 bf16 columns):
    [0:128]  hi(w * h)    [128:256] lo(w * h)
    [256]    hi(w)        [257]     lo(w)        [258] ones
"""

import numpy as np

N = 8192
D = 128
HID = 64
N_CORES = 8
ROWS = N // N_CORES          # 1024 output rows per core
JC = N // 128                # 64 contraction chunks of 128
NCOL = 259                   # moving columns per chunk (hi|lo|w_hi|w_lo|1)
ESHIFT = -4.0                # exp(e - 4): cancels exactly in the ratio, keeps
                             # w in a comfortable fp32/bf16 range

_cache = {}


def _install_axon_hooks_shim():
    """Provide antenv.axon_hooks if the image lacks it (trn_boot step 6).

    concourse.bass_utils imports it unconditionally when BASS_TRACE is set;
    without the shim that import crashes instead of degrading.
    """
    import contextlib
    import ctypes
    import sys
    import types

    try:
        import antenv.axon_hooks  # noqa: F401
        return
    except ImportError:
        pass

    so_path = "/opt/axon/libaxon_pjrt.so"

    def _make_hook():
        try:
            lib = ctypes.CDLL(so_path)
        except OSError:
            return None
        if not hasattr(lib, "axon_start_nrt_profile"):
            return None
        lib.axon_start_nrt_profile.argtypes = [
            ctypes.POINTER(ctypes.c_int64),
            ctypes.c_size_t,
        ]
        lib.axon_start_nrt_profile.restype = ctypes.c_int64
        lib.axon_stop_nrt_profile.argtypes = [ctypes.c_char_p]
        lib.axon_stop_nrt_profile.restype = ctypes.c_int64

        @contextlib.contextmanager
        def _hook(output_dir, device_ids):
            import jax

            jax.devices()
            if device_ids:
                ids = (ctypes.c_int64 * len(device_ids))(*device_ids)
                rc = lib.axon_start_nrt_profile(ids, len(device_ids))
            else:
                rc = lib.axon_start_nrt_profile(None, 0)
            if rc != 0:
                raise RuntimeError(f"axon_start_nrt_profile rc={rc}")
            try:
                yield
            finally:
                lib.axon_stop_nrt_profile(str(output_dir).encode())

        return _hook

    mod = types.ModuleType("antenv.axon_hooks")
    _holder = {"hook": _make_hook()}
    mod.set_axon_ntff_profile_hook = lambda h: _holder.__setitem__("hook", h)
    mod.get_axon_ntff_profile_hook = lambda: _holder["hook"]
    sys.modules["antenv.axon_hooks"] = mod
    try:
        import antenv

        antenv.axon_hooks = mod
    except ImportError:
        pass


def build_nc(enable_asserts=False):
    """Build + compile the per-core Bass program (identical on all 8 cores)."""
    from concourse import bacc, mybir, tile

    f32 = mybir.dt.float32
    bf16 = mybir.dt.bfloat16
    AF = mybir.ActivationFunctionType
    ALU = mybir.AluOpType

    nc = bacc.Bacc(
        "TRN2",
        target_bir_lowering=False,
        debug=False,
        enable_asserts=enable_asserts,
        num_devices=N_CORES,
    )
    gT = nc.dram_tensor("gT", [N, ROWS], bf16, kind="ExternalInput").ap()
    # hT split hi/lo in bf16 (host-side) so the MLP matmuls run at bf16 speed;
    # packed as [D, 2, N] so one DMA covers both halves
    hT2 = nc.dram_tensor("hT2", [D, 2, N], bf16, kind="ExternalInput").ap()
    hc = nc.dram_tensor("hc", [128, JC, D], f32, kind="ExternalInput").ap()
    # W1 columns doubled so the MLP writes z onto BOTH psum partition halves;
    # the hi-cast lives on partitions 0-63 and the lo-residual on 64-127,
    # which lets ONE matmul per chunk contract hi and lo against a stacked W2.
    W1h = nc.dram_tensor("W1h", [D, 2 * HID], bf16, kind="ExternalInput").ap()
    W1l = nc.dram_tensor("W1l", [D, 2 * HID], bf16, kind="ExternalInput").ap()
    b1 = nc.dram_tensor("b1", [2 * HID, 1], f32, kind="ExternalInput").ap()
    # W2s rows 0-63: [W2_hi | W2_lo]; rows 64-127: [W2_hi | 0], so
    # aT2.T @ W2s = [aThi@W2hi + aTlo@W2hi | aThi@W2lo] column-pair-wise.
    W2s = nc.dram_tensor("W2s", [2 * HID, 2], bf16, kind="ExternalInput").ap()
    b2 = nc.dram_tensor("b2", [1, 1], f32, kind="ExternalInput").ap()
    out = nc.dram_tensor("out", [ROWS, D], f32, kind="ExternalOutput").ap()

    with tile.TileContext(nc) as tc:
        with (
            tc.tile_pool(name="const", bufs=1) as cpool,
            tc.tile_pool(name="big", bufs=1) as bigpool,
            tc.tile_pool(name="gbuf", bufs=3) as gpool,
            tc.tile_pool(name="hpbuf", bufs=16) as hpool,
            tc.tile_pool(name="outbuf", bufs=3) as opool,
            tc.tile_pool(name="small", bufs=2) as spool,
        ):
            W1h_sb = cpool.tile([D, 2 * HID], bf16)
            nc.sync.dma_start(W1h_sb[:], W1h[:])
            W1l_sb = cpool.tile([D, 2 * HID], bf16)
            nc.sync.dma_start(W1l_sb[:], W1l[:])
            b1_sb = cpool.tile([2 * HID, 1], f32)
            nc.sync.dma_start(b1_sb[:], b1[:])
            W2s_sb = cpool.tile([2 * HID, 2], bf16)
            nc.sync.dma_start(W2s_sb[:], W2s[:])
            b2_sb = cpool.tile([1, 1], f32)
            nc.sync.dma_start(b2_sb[:], b2[:])
            ones_row = cpool.tile([1, 128], f32)
            nc.vector.memset(ones_row[:], 1.0)

            # h DMAs go on the gpsimd (SWDGE) queue so their dispatch doesn't
            # serialize with the G stream on the sync (HWDGE) queue; the MLP
            # h input is chunked so compute starts after the first quarter.
            NHCH = 8
            hT2_sb = bigpool.tile([D, 2, N], bf16)
            for q in range(NHCH):
                sl = slice(q * (N // NHCH), (q + 1) * (N // NHCH))
                nc.gpsimd.dma_start(hT2_sb[:, :, sl], hT2[:, :, sl])
            hc_sb = bigpool.tile([128, JC, D], f32)
            nc.gpsimd.dma_start(hc_sb[:], hc[:])

            # aT2 rows 0-63: bf16 hi of relu(h@W1+b1); rows 64-127: bf16 lo
            aT2 = bigpool.tile([2 * HID, N], bf16)

            w_sb = cpool.tile([128, JC], f32)
            # wtail[:, :, jc] = [w_hi, w_lo, 1] for chunk jc
            wtail = cpool.tile([128, 3, JC], bf16)
            nc.vector.memset(wtail[:, 2, :], 1.0)
            w_rem = cpool.tile([128, JC], f32)

            with tc.tile_pool(name="ps_pre", bufs=2, space="PSUM") as ps_pre:
                # ~4us of dummy matmuls on a zero tile: trips the PE HAM
                # activity monitor out of its 1.2 GHz cold state before the
                # real MLP arrives (no input deps, runs during the h DMA)
                warm = cpool.tile([128, 512], bf16)
                nc.vector.memset(warm[:], 0.0)
                pwarm = ps_pre.tile([128, 512], f32, tag="pwarm")
                for _ in range(32):
                    nc.tensor.matmul(
                        pwarm[:], warm[:, 0:128], warm[:], start=True, stop=True
                    )

                # ebias = b2 + ESHIFT broadcast to 128 partitions via a K=1
                # matmul (avoids any slow single-partition ops)
                pb2 = ps_pre.tile([128, 1], f32, tag="pb2")
                nc.tensor.matmul(pb2[:], ones_row[:], b2_sb[:], start=True,
                                 stop=True)
                ebias_sb = cpool.tile([128, 1], f32)
                nc.vector.tensor_scalar_add(ebias_sb[:], pb2[:], ESHIFT)

                af_all = bigpool.tile([2 * HID, N], f32)
                for nb in range(N // 512):
                    # a = h @ W1 in 3 bf16 terms: hi*hi + hi*lo + lo*hi
                    # (the dropped lo*lo term is ~2^-32 relative), written to
                    # BOTH psum partition halves via the doubled W1 columns
                    pa = ps_pre.tile([2 * HID, 512], f32, tag="pa")
                    sl = slice(nb * 512, (nb + 1) * 512)
                    nc.tensor.matmul(
                        pa[:], W1h_sb[:], hT2_sb[:, 0, sl], start=True,
                        stop=False,
                    )
                    nc.tensor.matmul(
                        pa[:], W1l_sb[:], hT2_sb[:, 0, sl], start=False,
                        stop=False,
                    )
                    nc.tensor.matmul(
                        pa[:], W1h_sb[:], hT2_sb[:, 1, sl], start=False,
                        stop=True,
                    )
                    nc.scalar.activation(
                        af_all[:, sl], pa[:], AF.Relu, bias=b1_sb[:]
                    )
                    # bf16 hi on rows 0-63; lo residual computed entirely on
                    # rows 64-127 (in-place: cast first, then subtract from
                    # the fp32 copy), batched per 4 blocks
                    if nb % 4 == 3:
                        bl = slice((nb - 3) * 512, (nb + 1) * 512)
                        nc.vector.tensor_copy(aT2[:, bl], af_all[:, bl])
                        nc.vector.scalar_tensor_tensor(
                            aT2[HID:, bl], af_all[HID:, bl], 1.0,
                            aT2[HID:, bl],
                            op0=ALU.mult, op1=ALU.subtract,
                        )

                # e laid out [128, 64]: partition = j within chunk, column =
                # chunk. ONE bf16 matmul per chunk: the stacked aT2 contracts
                # hi (rows 0-63) and lo (rows 64-127) against the stacked W2s
                # into a column pair summed after:
                # e = (aThi@W2hi + aTlo@W2hi) + aThi@W2lo.
                pe2 = ps_pre.tile([128, JC, 2], f32, tag="pe2")
                pe2s = cpool.tile([128, JC, 2], f32)
                pe_sum = cpool.tile([128, JC], f32)
                # combine -> exp -> w hi/lo per quarter so the first H' chunks
                # unblock the main loop while later scores still compute
                QW = JC // 4
                for q in range(4):
                    for c in range(q * QW, (q + 1) * QW):
                        nc.tensor.matmul(
                            pe2[:, c, :],
                            aT2[:, c * 128 : (c + 1) * 128],
                            W2s_sb[:],
                            start=True,
                            stop=True,
                        )
                    ql = slice(q * QW, (q + 1) * QW)
                    nc.vector.tensor_copy(pe2s[:, ql, :], pe2[:, ql, :])
                    nc.vector.tensor_tensor(
                        pe_sum[:, ql], pe2s[:, ql, 0], pe2s[:, ql, 1],
                        op=ALU.add,
                    )
                    nc.scalar.activation(
                        w_sb[:, ql], pe_sum[:, ql], AF.Exp, bias=ebias_sb[:]
                    )
                    nc.vector.tensor_copy(wtail[:, 0, ql], w_sb[:, ql])
                    nc.vector.scalar_tensor_tensor(
                        w_rem[:, ql], w_sb[:, ql], 1.0, wtail[:, 0, ql],
                        op0=ALU.mult, op1=ALU.subtract,
                    )
                    nc.vector.tensor_copy(wtail[:, 1, ql], w_rem[:, ql])

            # Main accumulation: acc[it] [128, NCOL] += G_tile.T @ H'_chunk.
            gTr = gT.rearrange("(a p) i -> p a i", p=128)  # [128, JC, ROWS]
            with tc.tile_pool(name="ps_acc", bufs=8, space="PSUM") as ps_acc:
                accs = [
                    ps_acc.tile([128, NCOL], f32, tag="acc", name=f"acc{i}")
                    for i in range(8)
                ]
                GRP = 8  # contraction chunks per DMA (2 MB transfers)
                for jg in range(JC // GRP):
                    gt = gpool.tile([128, GRP, ROWS], bf16, tag="gt")
                    nc.sync.dma_start(
                        gt[:], gTr[:, jg * GRP : (jg + 1) * GRP, :]
                    )
                    for jci in range(GRP):
                        jc = jg * GRP + jci
                        # just-in-time H' chunk build: 3 DVE ops
                        hp = hpool.tile([128, NCOL], bf16, tag="hp",
                                        name=f"hp{jc}")
                        nc.vector.tensor_scalar_mul(
                            hp[:, 0:128], hc_sb[:, jc, :], w_sb[:, jc : jc + 1]
                        )
                        nc.vector.scalar_tensor_tensor(
                            hp[:, 128:256], hc_sb[:, jc, :],
                            w_sb[:, jc : jc + 1], hp[:, 0:128],
                            op0=ALU.mult, op1=ALU.subtract,
                        )
                        nc.vector.tensor_copy(hp[:, 256:259], wtail[:, :, jc])
                        for it in range(8):
                            nc.tensor.matmul(
                                accs[it][:],
                                gt[:, jci, it * 128 : (it + 1) * 128],
                                hp[:],
                                start=(jc == 0),
                                stop=(jc == JC - 1),
                            )

                # epilogue: r = num / (sw + eps) batched across all 8 banks,
                # then 3 [128, 128] DVE ops + 1 DMA per bank.
                # (only one PSUM operand is legal per DVE op)
                # [128, it, (sw_hi, sw_lo, num)]: writes stay contiguous in the
                # innermost dim (non-contiguous DVE writes mis-lower on HW)
                tails = spool.tile([128, 8, 3], f32, tag="tails")
                for it in range(8):
                    nc.vector.tensor_copy(tails[:, it, :], accs[it][:, 256:259])
                swsum = spool.tile([128, 8], f32, tag="swsum")
                nc.vector.tensor_tensor(
                    swsum[:], tails[:, :, 0], tails[:, :, 1], op=ALU.add
                )
                swe = spool.tile([128, 8], f32, tag="swe")
                nc.vector.tensor_scalar_add(swe[:], swsum[:], 1e-30)
                rc8 = spool.tile([128, 8], f32, tag="rc8")
                nc.vector.reciprocal(rc8[:], swe[:])
                r8 = spool.tile([128, 8], f32, tag="r8")
                nc.vector.tensor_mul(r8[:], rc8[:], tails[:, :, 2])
                ot_all = opool.tile([128, 8, D], f32, tag="ot_all", bufs=1)
                for it in range(8):
                    shlo = opool.tile([128, D], f32, tag="shlo")
                    nc.vector.tensor_copy(shlo[:], accs[it][:, 128:256])
                    sh = opool.tile([128, D], f32, tag="sh")
                    nc.vector.tensor_tensor(
                        sh[:], accs[it][:, 0:128], shlo[:], op=ALU.add
                    )
                    nc.vector.tensor_scalar_mul(
                        ot_all[:, it, :], sh[:], r8[:, it : it + 1]
                    )
                nc.sync.dma_start(
                    out.rearrange("(a p) d -> p a d", p=128), ot_all[:]
                )

    nc.compile()
    return nc


def make_in_maps(graph_info, h, W1, b1, W2, b2):
    """Shard + lay out the full inputs for the 8 cores."""
    import ml_dtypes

    bf16 = ml_dtypes.bfloat16

    def hilo(x):
        hi = x.astype(bf16)
        lo = (x - hi.astype(np.float32)).astype(bf16)
        return hi, lo

    g = np.ascontiguousarray(graph_info, dtype=np.float32)
    GT = np.ascontiguousarray(g.T).astype(bf16)                # exact 0/1
    h = np.asarray(h, np.float32)
    hT = np.ascontiguousarray(h.T)                             # [D, N]
    hTh, hTl = hilo(hT)
    hT2 = np.ascontiguousarray(np.stack([hTh, hTl], axis=1))   # [D, 2, N]
    hcm = np.ascontiguousarray(
        h.reshape(JC, 128, D).transpose(1, 0, 2)               # [128, JC, D]
    )
    W1h, W1l = hilo(np.ascontiguousarray(np.asarray(W1, np.float32)))
    # doubled columns: the MLP psum carries z on both partition halves
    W1hd = np.ascontiguousarray(np.concatenate([W1h, W1h], axis=1))
    W1ld = np.ascontiguousarray(np.concatenate([W1l, W1l], axis=1))
    b1r = np.asarray(b1, np.float32).reshape(HID, 1)
    b1d = np.concatenate([b1r, b1r], axis=0)
    W2h, W2l = hilo(np.asarray(W2, np.float32).reshape(HID, 1))
    W2s = np.ascontiguousarray(
        np.concatenate(
            [np.concatenate([W2h, W2l], axis=1),
             np.concatenate([W2h, np.zeros_like(W2h)], axis=1)], axis=0
        )
    )
    b2r = np.asarray(b2, np.float32).reshape(1, 1)
    in_maps = []
    for c in range(N_CORES):
        in_maps.append(
            {
                "gT": GT[:, c * ROWS : (c + 1) * ROWS],
                "hT2": hT2,
                "hc": hcm,
                "W1h": W1hd,
                "W1l": W1ld,
                "b1": b1d,
                "W2s": W2s,
                "b2": b2r,
            }
        )
    return in_maps


def kernel(graph_info, h, W1, b1, W2, b2):
    _install_axon_hooks_shim()
    from concourse.bass_utils import run_bass_kernel_spmd

    if "nc" not in _cache:
        _cache["nc"] = build_nc()
    nc = _cache["nc"]

    in_maps = make_in_maps(graph_info, h, W1, b1, W2, b2)
    res = run_bass_kernel_spmd(nc, in_maps, list(range(N_CORES)))
    return np.concatenate([res.results[c]["out"] for c in range(N_CORES)], axis=0)


# revision 50
# speedup vs baseline: 2.0245x; 1.0001x over previous
"""Trainium2 Bass kernel for nn_Att_mlp_softmax (GNN message passing).

Reference computation:
    e = relu(h @ W1 + b1) @ W2 + b2                       # [N, 1] per-node score
    att = softmax(where(G > 0, e.T broadcast, -9e15))     # row-wise over neighbors
    out = (G.sum(-1))[:, None] * (att @ h)                # degree-rescaled aggregation

Because the pre-softmax score of entry (i, j) depends only on column j, the
masked softmax collapses algebraically:
    att[i, j] = G[i, j] * w[j] / sum_j G[i, j] * w[j],  w = exp(e - c)
so with H' = [w * h | w | 1] (N x 130):
    Y = G @ H'
    out = Y[:, 129] * Y[:, :128] / Y[:, 128]
One big [N, N] x [N, 130] matmul replaces the N^2 softmax entirely.

Precision/perf: G is an exact 0/1 mask, so it is streamed in bf16 losslessly
(half the HBM traffic, 1 cycle/row matmul, FWL weight loads). H' is split
hi/lo into two bf16 columns per logical column (err ~2^-16), accumulated in
fp32 PSUM — near-fp32 accuracy at bf16 speed. H' chunks are built just-in-time
inside the main loop (3 fused DVE ops per chunk) so the build fully overlaps
the matmul stream.

Distribution: G is row-sharded across 8 NeuronCores (1024 rows each); h and the
MLP weights are replicated. Each core's G shard is passed pre-transposed
([N, 1024], contraction dim major) so SBUF tiles have the contraction dim on
partitions with fully contiguous DMA lines. h is passed twice: d-major bf16
hi/lo (hT2, for the MLP contraction) and chunk-major natural fp32 (hc, for the
H' build). No collectives needed.

Moving-operand layout per contraction chunk jc (259 bf16 columns):
    [0:128]  hi(w * h)    [128:256] lo(w * h)
    [256]    hi(w)        [257]     lo(w)        [258] ones
"""

import numpy as np

N = 8192
D = 128
HID = 64
N_CORES = 8
ROWS = N // N_CORES          # 1024 output rows per core
JC = N // 128                # 64 contraction chunks of 128
NCOL = 259                   # moving columns per chunk (hi|lo|w_hi|w_lo|1)
ESHIFT = -4.0                # exp(e - 4): cancels exactly in the ratio, keeps
                             # w in a comfortable fp32/bf16 range

_cache = {}


def _install_axon_hooks_shim():
    """Provide antenv.axon_hooks if the image lacks it (trn_boot step 6).

    concourse.bass_utils imports it unconditionally when BASS_TRACE is set;
    without the shim that import crashes instead of degrading.
    """
    import contextlib
    import ctypes
    import sys
    import types

    try:
        import antenv.axon_hooks  # noqa: F401
        return
    except ImportError:
        pass

    so_path = "/opt/axon/libaxon_pjrt.so"

    def _make_hook():
        try:
            lib = ctypes.CDLL(so_path)
        except OSError:
            return None
        if not hasattr(lib, "axon_start_nrt_profile"):
            return None
        lib.axon_start_nrt_profile.argtypes = [
            ctypes.POINTER(ctypes.c_int64),
            ctypes.c_size_t,
        ]
        lib.axon_start_nrt_profile.restype = ctypes.c_int64
        lib.axon_stop_nrt_profile.argtypes = [ctypes.c_char_p]
        lib.axon_stop_nrt_profile.restype = ctypes.c_int64

        @contextlib.contextmanager
        def _hook(output_dir, device_ids):
            import jax

            jax.devices()
            if device_ids:
                ids = (ctypes.c_int64 * len(device_ids))(*device_ids)
                rc = lib.axon_start_nrt_profile(ids, len(device_ids))
            else:
                rc = lib.axon_start_nrt_profile(None, 0)
            if rc != 0:
                raise RuntimeError(f"axon_start_nrt_profile rc={rc}")
            try:
                yield
            finally:
                lib.axon_stop_nrt_profile(str(output_dir).encode())

        return _hook

    mod = types.ModuleType("antenv.axon_hooks")
    _holder = {"hook": _make_hook()}
    mod.set_axon_ntff_profile_hook = lambda h: _holder.__setitem__("hook", h)
    mod.get_axon_ntff_profile_hook = lambda: _holder["hook"]
    sys.modules["antenv.axon_hooks"] = mod
    try:
        import antenv

        antenv.axon_hooks = mod
    except ImportError:
        pass


def build_nc(enable_asserts=False):
    """Build + compile the per-core Bass program (identical on all 8 cores)."""
    from concourse import bacc, mybir, tile

    f32 = mybir.dt.float32
    bf16 = mybir.dt.bfloat16
    AF = mybir.ActivationFunctionType
    ALU = mybir.AluOpType

    nc = bacc.Bacc(
        "TRN2",
        target_bir_lowering=False,
        debug=False,
        enable_asserts=enable_asserts,
        num_devices=N_CORES,
    )
    gT = nc.dram_tensor("gT", [N, ROWS], bf16, kind="ExternalInput").ap()
    # hT split hi/lo in bf16 (host-side) so the MLP matmuls run at bf16 speed;
    # packed as [D, 2, N] so one DMA covers both halves
    hT2 = nc.dram_tensor("hT2", [D, 2, N], bf16, kind="ExternalInput").ap()
    hc = nc.dram_tensor("hc", [128, JC, D], f32, kind="ExternalInput").ap()
    # W1 columns doubled so the MLP writes z onto BOTH psum partition halves;
    # the hi-cast lives on partitions 0-63 and the lo-residual on 64-127,
    # which lets ONE matmul per chunk contract hi and lo against a stacked W2.
    W1h = nc.dram_tensor("W1h", [D, 2 * HID], bf16, kind="ExternalInput").ap()
    W1l = nc.dram_tensor("W1l", [D, 2 * HID], bf16, kind="ExternalInput").ap()
    b1 = nc.dram_tensor("b1", [2 * HID, 1], f32, kind="ExternalInput").ap()
    # W2s rows 0-63: [W2_hi | W2_lo]; rows 64-127: [W2_hi | 0], so
    # aT2.T @ W2s = [aThi@W2hi + aTlo@W2hi | aThi@W2lo] column-pair-wise.
    W2s = nc.dram_tensor("W2s", [2 * HID, 2], bf16, kind="ExternalInput").ap()
    b2 = nc.dram_tensor("b2", [1, 1], f32, kind="ExternalInput").ap()
    out = nc.dram_tensor("out", [ROWS, D], f32, kind="ExternalOutput").ap()

    with tile.TileContext(nc) as tc:
        with (
            tc.tile_pool(name="const", bufs=1) as cpool,
            tc.tile_pool(name="big", bufs=1) as bigpool,
            tc.tile_pool(name="gbuf", bufs=3) as gpool,
            tc.tile_pool(name="hpbuf", bufs=16) as hpool,
            tc.tile_pool(name="outbuf", bufs=3) as opool,
            tc.tile_pool(name="small", bufs=2) as spool,
        ):
            W1h_sb = cpool.tile([D, 2 * HID], bf16)
            nc.sync.dma_start(W1h_sb[:], W1h[:])
            W1l_sb = cpool.tile([D, 2 * HID], bf16)
            nc.sync.dma_start(W1l_sb[:], W1l[:])
            b1_sb = cpool.tile([2 * HID, 1], f32)
            nc.sync.dma_start(b1_sb[:], b1[:])
            W2s_sb = cpool.tile([2 * HID, 2], bf16)
            nc.sync.dma_start(W2s_sb[:], W2s[:])
            b2_sb = cpool.tile([1, 1], f32)
            nc.sync.dma_start(b2_sb[:], b2[:])
            ones_row = cpool.tile([1, 128], f32)
            nc.vector.memset(ones_row[:], 1.0)

            # h DMAs go on the gpsimd (SWDGE) queue so their dispatch doesn't
            # serialize with the G stream on the sync (HWDGE) queue; the MLP
            # h input is chunked so compute starts after the first quarter.
            NHCH = 8
            hT2_sb = bigpool.tile([D, 2, N], bf16)
            for q in range(NHCH):
                sl = slice(q * (N // NHCH), (q + 1) * (N // NHCH))
                nc.gpsimd.dma_start(hT2_sb[:, :, sl], hT2[:, :, sl])
            hc_sb = bigpool.tile([128, JC, D], f32)
            nc.gpsimd.dma_start(hc_sb[:], hc[:])

            # aT2 rows 0-63: bf16 hi of relu(h@W1+b1); rows 64-127: bf16 lo
            aT2 = bigpool.tile([2 * HID, N], bf16)

            w_sb = cpool.tile([128, JC], f32)
            # wtail[:, :, jc] = [w_hi, w_lo, 1] for chunk jc
            wtail = cpool.tile([128, 3, JC], bf16)
            nc.vector.memset(wtail[:, 2, :], 1.0)
            w_rem = cpool.tile([128, JC], f32)

            with tc.tile_pool(name="ps_pre", bufs=2, space="PSUM") as ps_pre:
                # ~4us of dummy matmuls on a zero tile: trips the PE HAM
                # activity monitor out of its 1.2 GHz cold state before the
                # real MLP arrives (no input deps, runs during the h DMA)
                warm = cpool.tile([128, 512], bf16)
                nc.vector.memset(warm[:], 0.0)
                pwarm = ps_pre.tile([128, 512], f32, tag="pwarm")
                for _ in range(32):
                    nc.tensor.matmul(
                        pwarm[:], warm[:, 0:128], warm[:], start=True, stop=True
                    )

                # ebias = b2 + ESHIFT broadcast to 128 partitions via a K=1
                # matmul (avoids any slow single-partition ops)
                pb2 = ps_pre.tile([128, 1], f32, tag="pb2")
                nc.tensor.matmul(pb2[:], ones_row[:], b2_sb[:], start=True,
                                 stop=True)
                ebias_sb = cpool.tile([128, 1], f32)
                nc.vector.tensor_scalar_add(ebias_sb[:], pb2[:], ESHIFT)

                af_all = bigpool.tile([2 * HID, N], f32)
                for nb in range(N // 512):
                    # a = h @ W1 in 3 bf16 terms: hi*hi + hi*lo + lo*hi
                    # (the dropped lo*lo term is ~2^-32 relative), written to
                    # BOTH psum partition halves via the doubled W1 columns
                    pa = ps_pre.tile([2 * HID, 512], f32, tag="pa")
                    sl = slice(nb * 512, (nb + 1) * 512)
                    nc.tensor.matmul(
                        pa[:], W1h_sb[:], hT2_sb[:, 0, sl], start=True,
                        stop=False,
                    )
                    nc.tensor.matmul(
                        pa[:], W1l_sb[:], hT2_sb[:, 0, sl], start=False,
                        stop=False,
                    )
                    nc.tensor.matmul(
                        pa[:], W1h_sb[:], hT2_sb[:, 1, sl], start=False,
                        stop=True,
                    )
                    nc.scalar.activation(
                        af_all[:, sl], pa[:], AF.Relu, bias=b1_sb[:]
                    )
                    # bf16 hi on rows 0-63; lo residual computed entirely on
                    # rows 64-127 (in-place: cast first, then subtract from
                    # the fp32 copy), batched per 4 blocks
                    if nb % 4 == 3:
                        bl = slice((nb - 3) * 512, (nb + 1) * 512)
                        nc.vector.tensor_copy(aT2[:, bl], af_all[:, bl])
                        nc.vector.scalar_tensor_tensor(
                            aT2[HID:, bl], af_all[HID:, bl], 1.0,
                            aT2[HID:, bl],
                            op0=ALU.mult, op1=ALU.subtract,
                        )

                # e laid out [128, 64]: partition = j within chunk, column =
                # chunk. ONE bf16 matmul per chunk: the stacked aT2 contracts
                # hi (rows 0-63) and lo (rows 64-127) against the stacked W2s
                # into a column pair summed after:
                # e = (aThi@W2hi + aTlo@W2hi) + aThi@W2lo.
                pe2 = ps_pre.tile([128, JC, 2], f32, tag="pe2")
                pe2s = cpool.tile([128, JC, 2], f32)
                pe_sum = cpool.tile([128, JC], f32)
                # combine -> exp -> w hi/lo per quarter so the first H' chunks
                # unblock the main loop while later scores still compute
                QW = JC // 4
                for q in range(4):
                    for c in range(q * QW, (q + 1) * QW):
                        nc.tensor.matmul(
                            pe2[:, c, :],
                            aT2[:, c * 128 : (c + 1) * 128],
                            W2s_sb[:],
                            start=True,
                            stop=True,
                        )
                    ql = slice(q * QW, (q + 1) * QW)
                    nc.vector.tensor_copy(pe2s[:, ql, :], pe2[:, ql, :])
                    nc.vector.tensor_tensor(
                        pe_sum[:, ql], pe2s[:, ql, 0], pe2s[:, ql, 1],
                        op=ALU.add,
                    )
                    nc.scalar.activation(
                        w_sb[:, ql], pe_sum[:, ql], AF.Exp, bias=ebias_sb[:]
                    )
                    nc.vector.tensor_copy(wtail[:, 0, ql], w_sb[:, ql])
                    nc.vector.scalar_tensor_tensor(
                        w_rem[:, ql], w_sb[:, ql], 1.0, wtail[:, 0, ql],
                        op0=ALU.mult, op1=ALU.subtract,
                    )
                    nc.vector.tensor_copy(wtail[:, 1, ql], w_rem[:, ql])

            # Main accumulation: acc[it] [128, NCOL] += G_tile.T @ H'_chunk.
            gTr = gT.rearrange("(a p) i -> p a i", p=128)  # [128, JC, ROWS]
            with tc.tile_pool(name="ps_acc", bufs=8, space="PSUM") as ps_acc:
                accs = [
                    ps_acc.tile([128, NCOL], f32, tag="acc", name=f"acc{i}")
                    for i in range(8)
                ]
                GRP = 8  # contraction chunks per DMA (2 MB transfers)
                for jg in range(JC // GRP):
                    gt = gpool.tile([128, GRP, ROWS], bf16, tag="gt")
                    nc.sync.dma_start(
                        gt[:], gTr[:, jg * GRP : (jg + 1) * GRP, :]
                    )
                    for jci in range(GRP):
                        jc = jg * GRP + jci
                        # just-in-time H' chunk build: 3 DVE ops
                        hp = hpool.tile([128, NCOL], bf16, tag="hp",
                                        name=f"hp{jc}")
                        nc.vector.tensor_scalar_mul(
                            hp[:, 0:128], hc_sb[:, jc, :], w_sb[:, jc : jc + 1]
                        )
                        nc.vector.scalar_tensor_tensor(
                            hp[:, 128:256], hc_sb[:, jc, :],
                            w_sb[:, jc : jc + 1], hp[:, 0:128],
                            op0=ALU.mult, op1=ALU.subtract,
                        )
                        nc.vector.tensor_copy(hp[:, 256:259], wtail[:, :, jc])
                        for it in range(8):
                            nc.tensor.matmul(
                                accs[it][:],
                                gt[:, jci, it * 128 : (it + 1) * 128],
                                hp[:],
                                start=(jc == 0),
                                stop=(jc == JC - 1),
                            )

                # epilogue: r = num / (sw + eps) batched across all 8 banks,
                # then 3 [128, 128] DVE ops + 1 DMA per bank.
                # (only one PSUM operand is legal per DVE op)
                # [128, it, (sw_hi, sw_lo, num)]: writes stay contiguous in the
                # innermost dim (non-contiguous DVE writes mis-lower on HW)
                tails = spool.tile([128, 8, 3], f32, tag="tails")
                for it in range(8):
                    nc.vector.tensor_copy(tails[:, it, :], accs[it][:, 256:259])
                swsum = spool.tile([128, 8], f32, tag="swsum")
                nc.vector.tensor_tensor(
                    swsum[:], tails[:, :, 0], tails[:, :, 1], op=ALU.add
                )
                swe = spool.tile([128, 8], f32, tag="swe")
                nc.vector.tensor_scalar_add(swe[:], swsum[:], 1e-30)
                rc8 = spool.tile([128, 8], f32, tag="rc8")
                nc.vector.reciprocal(rc8[:], swe[:])
                r8 = spool.tile([128, 8], f32, tag="r8")
                nc.vector.tensor_mul(r8[:], rc8[:], tails[:, :, 2])
                ot_all = opool.tile([128, 8, D], f32, tag="ot_all", bufs=1)
                for it in range(8):
                    shlo = opool.tile([128, D], f32, tag="shlo")
                    nc.vector.tensor_copy(shlo[:], accs[it][:, 128:256])
                    sh = opool.tile([128, D], f32, tag="sh")
                    nc.vector.tensor_tensor(
                        sh[:], accs[it][:, 0:128], shlo[:], op=ALU.add
                    )
                    nc.vector.tensor_scalar_mul(
                        ot_all[:, it, :], sh[:], r8[:, it : it + 1]
                    )
                nc.sync.dma_start(
                    out.rearrange("(a p) d -> p a d", p=128), ot_all[:]
                )

    nc.compile()
    return nc


def make_in_maps(graph_info, h, W1, b1, W2, b2):
    """Shard + lay out the full inputs for the 8 cores."""
    import ml_dtypes

    bf16 = ml_dtypes.bfloat16

    def hilo(x):
        hi = x.astype(bf16)
        lo = (x - hi.astype(np.float32)).astype(bf16)
        return hi, lo

    g = np.ascontiguousarray(graph_info, dtype=np.float32)
    GT = np.ascontiguousarray(g.T).astype(bf16)                # exact 0/1
    h = np.asarray(h, np.float32)
    hT = np.ascontiguousarray(h.T)                             # [D, N]
    hTh, hTl = hilo(hT)
    hT2 = np.ascontiguousarray(np.stack([hTh, hTl], axis=1))   # [D, 2, N]
    hcm = np.ascontiguousarray(
        h.reshape(JC, 128, D).transpose(1, 0, 2)               # [128, JC, D]
    )
    W1h, W1l = hilo(np.ascontiguousarray(np.asarray(W1, np.float32)))
    # doubled columns: the MLP psum carries z on both partition halves
    W1hd = np.ascontiguousarray(np.concatenate([W1h, W1h], axis=1))
    W1ld = np.ascontiguousarray(np.concatenate([W1l, W1l], axis=1))
    b1r = np.asarray(b1, np.float32).reshape(HID, 1)
    b1d = np.concatenate([b1r, b1r], axis=0)
    W2h, W2l = hilo(np.asarray(W2, np.float32).reshape(HID, 1))
    W2s = np.ascontiguousarray(
        np.concatenate(
            [np.concatenate([W2h, W2l], axis=1),
             np.concatenate([W2h, np.zeros_like(W2h)], axis=1)], axis=0
        )
    )
    b2r = np.asarray(b2, np.float32).reshape(1, 1)
    in_maps = []
    for c in range(N_CORES):
        in_maps.append(
            {
                "gT": GT[:, c * ROWS : (c + 1) * ROWS],
                "hT2": hT2,
                "hc": hcm,
                "W1h": W1hd,
                "W1l": W1ld,
                "b1": b1d,
                "W2s": W2s,
                "b2": b2r,
            }
        )
    return in_maps


def kernel(graph_info, h, W1, b1, W2, b2):
    _install_axon_hooks_shim()
    from concourse.bass_utils import run_bass_kernel_spmd

    if "nc" not in _cache:
        _cache["nc"] = build_nc()
    nc = _cache["nc"]

    in_maps = make_in_maps(graph_info, h, W1, b1, W2, b2)
    res = run_bass_kernel_spmd(nc, in_maps, list(range(N_CORES)))
    return np.concatenate([res.results[c]["out"] for c in range(N_CORES)], axis=0)
